# revision 1
# baseline (speedup 1.0000x reference)
"""DGL-style 2-layer GCN encoder on 8 Trainium2 NeuronCores.

Strategy (graph/data parallel, dst-sharded):
  - Nodes padded 100000 -> 102400 and split into 8 shards of 12800 (100 blocks
    of 128 nodes per core). Each core owns the aggregation for its dst shard.
  - Algebraic reformulation (linear ops commute):
        layer(h) = (segsum((h*norm_s)[src] -> dst) * norm_d) @ W + b
                 = segsum(((h*norm_s) @ W)[src] -> dst) * norm_d + b
    so the dense matmul runs over nodes (PE-friendly) and only the sparse
    segment-sum runs over edges.
  - Sparse segment-sum per core: edges sorted by (block-group, src-bank,
    dst-block); gathered 512B rows via dma_gather (int16 indices, 4 banks of
    25600 rows); scatter-sum via one-hot matmuls on PE (lhsT = one-hot built
    by DVE is_equal against an iota row, rhs = gathered rows), accumulated in
    PSUM per dst block, flushed with the norm_d scale fused on ACT.
  - No device collectives: three small NEFFs (dense t1; one GCN layer,
    invoked twice). The host only concatenates/transposes shards between
    NEFF invocations (no host FLOPs on the model math).
"""

import os
import sys
import time
import types
from contextlib import ExitStack

import numpy as np

sys.path.insert(0, "/opt/trn_rl_repo")

import concourse.bacc as bacc
import concourse.bass as bass
import concourse.tile as tile
from concourse import mybir
from concourse import bass_utils

F32 = mybir.dt.float32
F16 = mybir.dt.float16
I16 = mybir.dt.int16
SPARSE_FP32 = bool(int(os.environ.get("GCN_FP32", "0")))
TDT = F32 if SPARSE_FP32 else F16          # device dtype of the gathered table
TNP = np.float32 if SPARSE_FP32 else np.float16
AF = mybir.ActivationFunctionType
ALU = mybir.AluOpType

P = 128
D = 128
N_RAW = 100000
N_CORES = 8
NPC = 12800            # nodes per core
NB = NPC // P          # 100 dst blocks per core
N = NPC * N_CORES      # 102400 padded node count
BANKS = 4
BANK_ROWS = N // BANKS  # 25600 (< int16 max)
GROUP_BLOCKS = 4       # dst blocks per PSUM group (share one PSUM bank tile)
NGROUPS = NB // GROUP_BLOCKS
MAX_CALL_TILES = 24    # max tiles (128 idxs each) per dma_gather call

TRACE = bool(int(os.environ.get("GCN_TRACE", "0")))
LAST_EXEC_NS = []      # [(name, exec_time_ns)] of the most recent kernel() call

_cache = {}


def _install_ntff_shim():
    if "antenv.axon_hooks" in sys.modules:
        return
    try:
        from trn_agent_boot.trn_boot import _ntff_profile_via_ctypes

        hook = _ntff_profile_via_ctypes("/opt/axon/libaxon_pjrt.so")
        mod = types.ModuleType("antenv.axon_hooks")
        mod.get_axon_ntff_profile_hook = lambda: hook
        mod.set_axon_ntff_profile_hook = lambda h: None
        sys.modules["antenv.axon_hooks"] = mod
        bass_utils.upload_artifacts = lambda tmpdir: "local://" + tmpdir
    except Exception:
        pass


# --------------------------------------------------------------------------
# device programs
# --------------------------------------------------------------------------

def _emit_norm(nc, sb, deg_ap, nblk):
    """norm = clip(deg,1)^-0.5 with two Newton refinements; [P, nblk] tile."""
    deg = sb.tile([P, nblk], F32, tag="norm_deg")
    nc.sync.dma_start(deg[:], deg_ap[:])
    dcl = sb.tile([P, nblk], F32, tag="norm_dcl")
    nc.vector.tensor_scalar_max(dcl[:], deg[:], 1.0)
    s = sb.tile([P, nblk], F32, tag="norm_s")
    nc.scalar.activation(s[:], dcl[:], AF.Sqrt)
    r = sb.tile([P, nblk], F32, tag="norm_r0")
    nc.vector.reciprocal(r[:], s[:])
    for it in range(2):
        t0 = sb.tile([P, nblk], F32, tag=f"norm_t{it}a")
        nc.vector.tensor_mul(t0[:], r[:], r[:])
        t1 = sb.tile([P, nblk], F32, tag=f"norm_t{it}b")
        nc.vector.tensor_mul(t1[:], t0[:], dcl[:])
        t2 = sb.tile([P, nblk], F32, tag=f"norm_t{it}c")
        nc.vector.tensor_scalar(t2[:], t1[:], -0.5, 1.5, ALU.mult, ALU.add)
        r2 = sb.tile([P, nblk], F32, tag=f"norm_r{it + 1}")
        nc.vector.tensor_mul(r2[:], r[:], t2[:])
        r = r2
    return r


def _emit_dense_tail(nc, ctx, tc, sb, src_sb, w_t, ident_t, out_dram, nblk,
                     out_dt=F32):
    """out_dram[P, nblk*P] (feat-major) = W.T @ src_sb.T per node block.

    src_sb: SBUF [P, nblk, P] node-major (partition=node%128).
    """
    pt = ctx.enter_context(tc.tile_pool(name="tailpsum", bufs=4, space="PSUM"))
    pm = ctx.enter_context(tc.tile_pool(name="tailmm", bufs=2, space="PSUM"))
    st = ctx.enter_context(tc.tile_pool(name="tailsb", bufs=3))
    CH = 4  # blocks per dense matmul (N=512)
    for c in range(nblk // CH):
        rhs = st.tile([P, CH * P], F32, tag="rhs")
        for k in range(CH):
            d = c * CH + k
            tp = pt.tile([P, P], F32, space="PSUM")
            nc.tensor.transpose(out=tp[:], in_=src_sb[:, d, :], identity=ident_t[:])
            nc.vector.tensor_copy(rhs[:, k * P:(k + 1) * P], tp[:])
        mm = pm.tile([P, CH * P], F32, space="PSUM")
        nc.tensor.matmul(out=mm[:], lhsT=w_t[:], rhs=rhs[:], start=True, stop=True)
        stg = st.tile([P, CH * P], out_dt, tag="stg")
        nc.scalar.activation(stg[:], mm[:], AF.Copy)
        nc.sync.dma_start(out_dram[:, c * CH * P:(c + 1) * CH * P], stg[:])


def _build_dense(n_nodes=NPC):
    """NEFF-A: t1.T = ((feat*norm_s) @ W1).T for this core's shard."""
    nblk = n_nodes // P
    nc = bacc.Bacc("TRN2", target_bir_lowering=False, debug=False,
                   enable_asserts=False, num_devices=N_CORES)
    feat = nc.dram_tensor("feat", [n_nodes, D], F32, kind="ExternalInput").ap()
    deg_out = nc.dram_tensor("deg_out", [P, nblk], F32, kind="ExternalInput").ap()
    w1 = nc.dram_tensor("w1", [D, D], F32, kind="ExternalInput").ap()
    ident = nc.dram_tensor("ident", [P, P], F32, kind="ExternalInput").ap()
    t1t = nc.dram_tensor("t1t", [P, n_nodes], TDT, kind="ExternalOutput").ap()

    with tile.TileContext(nc) as tc, ExitStack() as ctx:
        sb = ctx.enter_context(tc.tile_pool(name="sb", bufs=1))
        ident_t = sb.tile([P, P], F32)
        nc.sync.dma_start(ident_t[:], ident[:])
        w_t = sb.tile([P, P], F32)
        nc.sync.dma_start(w_t[:], w1[:])
        norm_s = _emit_norm(nc, sb, deg_out, nblk)

        fsb = sb.tile([P, nblk, P], F32)
        nc.sync.dma_start(fsb[:], feat.rearrange("(d p) f -> p d f", p=P))
        fs = sb.tile([P, nblk, P], F32)
        nc.vector.tensor_mul(
            fs[:], fsb[:], norm_s[:].unsqueeze(2).to_broadcast([P, nblk, P])
        )
        _emit_dense_tail(nc, ctx, tc, sb, fs, w_t, ident_t, t1t, nblk,
                         out_dt=TDT)
    nc.compile()
    return nc


def _build_layer(tiles_per_cell, n_nodes_tab=N, nblk=NB, banks=BANKS,
                 bank_rows=BANK_ROWS, group_blocks=GROUP_BLOCKS):
    """NEFF-X: one GCN layer for this core's dst shard.

    tiles_per_cell: [banks][nblk] ints, identical for all cores.
    outputs:
      out1: [nblk*P, D] node-major  = segsum(table[src]) * norm_d + bias
      tnt:  [P, nblk*P] feat-major  = (relu(out1) * norm_s) @ Wn, transposed
    """
    ngroups = nblk // group_blocks
    tot_tiles = sum(sum(r) for r in tiles_per_cell)
    nc = bacc.Bacc("TRN2", target_bir_lowering=False, debug=False,
                   enable_asserts=False, num_devices=N_CORES,
                   num_swdge_queues=2)
    table = nc.dram_tensor("table", [n_nodes_tab, D], TDT, kind="ExternalInput").ap()
    idx16 = nc.dram_tensor("idx16", [P, tot_tiles * 8], I16, kind="ExternalInput").ap()
    dstval = nc.dram_tensor("dstval", [P, tot_tiles], F32, kind="ExternalInput").ap()
    deg_in = nc.dram_tensor("deg_in", [P, nblk], F32, kind="ExternalInput").ap()
    deg_out = nc.dram_tensor("deg_out", [P, nblk], F32, kind="ExternalInput").ap()
    wn = nc.dram_tensor("wn", [D, D], F32, kind="ExternalInput").ap()
    bias = nc.dram_tensor("bias", [P, D], F32, kind="ExternalInput").ap()
    iota_c = nc.dram_tensor("iota_c", [P, P], F32, kind="ExternalInput").ap()
    ident = nc.dram_tensor("ident", [P, P], F32, kind="ExternalInput").ap()
    out1 = nc.dram_tensor("out1", [nblk * P, D], F32, kind="ExternalOutput").ap()
    tnt = nc.dram_tensor("tnt", [P, nblk * P], TDT, kind="ExternalOutput").ap()

    with tile.TileContext(nc) as tc, ExitStack() as ctx:
        sb = ctx.enter_context(tc.tile_pool(name="sb", bufs=1))
        ident_t = sb.tile([P, P], F32)
        nc.sync.dma_start(ident_t[:], ident[:])
        iota_t = sb.tile([P, P], F32)
        nc.sync.dma_start(iota_t[:], iota_c[:])
        w_t = sb.tile([P, P], F32)
        nc.sync.dma_start(w_t[:], wn[:])
        bias_t = sb.tile([P, P], F32)
        nc.sync.dma_start(bias_t[:], bias[:])
        norm_d = _emit_norm(nc, sb, deg_in, nblk)
        norm_s = _emit_norm(nc, sb, deg_out, nblk)

        aggs = sb.tile([P, nblk, P], F32)   # norm_d-scaled aggregation
        out1_sb = sb.tile([P, nblk, P], F32)

        with ExitStack() as sctx:
            gp = sctx.enter_context(tc.tile_pool(name="gather", bufs=6))
            sp = sctx.enter_context(tc.tile_pool(name="onehot", bufs=6))
            ip = sctx.enter_context(tc.tile_pool(name="idxpool", bufs=8))
            pp = sctx.enter_context(
                tc.tile_pool(name="cellpsum", bufs=6, space="PSUM"))

            # one PSUM accumulation group per (group, bank-tile): start on the
            # very first matmul of the group, stop on the very last.  Each
            # block has >=1 tile so every 128-col slice gets written.
            group_mm_total = [
                sum(tiles_per_cell[b][d]
                    for b in range(banks)
                    for d in range(g * group_blocks, (g + 1) * group_blocks))
                for g in range(ngroups)
            ]

            tile_off = 0  # global tile cursor (== order of the edge stream)
            call_no = 0
            for g in range(ngroups):
                blocks = range(g * group_blocks, (g + 1) * group_blocks)
                # one PSUM bank tile holds all blocks of the group
                gps = pp.tile([P, group_blocks * P], F32, space="PSUM",
                              name="cellpsum", tag="cellpsum")
                psums = {d: gps[:, (d % group_blocks) * P:
                                (d % group_blocks + 1) * P] for d in blocks}
                mm_done = 0
                # per-block remaining-tile counters to time the flushes
                blk_left = {d: sum(tiles_per_cell[b][d] for b in range(banks))
                            for d in blocks}
                for b in range(banks):
                    # tiles of this (group, bank), split into <=MAX_CALL_TILES calls
                    gb_tiles = []  # (d, j, call_rel_pos) assembled per call
                    for d in blocks:
                        for j in range(tiles_per_cell[b][d]):
                            gb_tiles.append((d, j))
                    pos = 0
                    while pos < len(gb_tiles):
                        call = gb_tiles[pos:pos + MAX_CALL_TILES]
                        ct = len(call)
                        ni = ct * P
                        idx_t = ip.tile([P, ct * 8], I16, tag="idx")
                        nc.sync.dma_start(
                            idx_t[:], idx16[:, tile_off * 8:(tile_off + ct) * 8])
                        dv_t = ip.tile([P, ct], F32, tag="dv")
                        nc.scalar.dma_start(
                            dv_t[:], dstval[:, tile_off:tile_off + ct])
                        G = gp.tile([P, ct, P], TDT, tag="G")
                        nc.gpsimd.dma_gather(
                            G[:], table[b * bank_rows:(b + 1) * bank_rows, :],
                            idx_t[:], ni, ni, D, single_packet=False,
                            queue_num=call_no % 2)
                        call_no += 1
                        S = sp.tile([P, ct, P], TDT, tag="S")
                        nc.vector.tensor_tensor(
                            out=S[:],
                            in0=dv_t[:].unsqueeze(2).to_broadcast([P, ct, P]),
                            in1=iota_t[:].unsqueeze(1).to_broadcast([P, ct, P]),
                            op=ALU.is_equal)
                        for k, (d, j) in enumerate(call):
                            is_start = mm_done == 0
                            is_stop = mm_done == group_mm_total[g] - 1
                            nc.tensor.matmul(
                                out=psums[d][:], lhsT=S[:, k, :], rhs=G[:, k, :],
                                start=is_start, stop=is_stop)
                            mm_done += 1
                            blk_left[d] -= 1
                            if is_stop:
                                # group complete: flush every block's slice
                                # with the norm_d scale fused on ACT
                                for d2 in blocks:
                                    assert blk_left[d2] == 0, (g, d2)
                                    nc.scalar.activation(
                                        out=aggs[:, d2, :], in_=psums[d2][:],
                                        func=AF.Copy,
                                        scale=norm_d[:, d2:d2 + 1])
                        tile_off += ct
                        pos += ct

        # out1 = aggs + bias
        nc.vector.tensor_add(
            out1_sb[:], aggs[:],
            bias_t[:].unsqueeze(1).to_broadcast([P, nblk, P]))
        nc.sync.dma_start(out1.rearrange("(d p) f -> p d f", p=P), out1_sb[:])

        # hs = relu(out1 * norm_s)  (norm_s >= 0 so scale commutes with relu)
        for d in range(nblk):
            nc.scalar.activation(
                out=aggs[:, d, :], in_=out1_sb[:, d, :], func=AF.Relu,
                scale=norm_s[:, d:d + 1])
        _emit_dense_tail(nc, ctx, tc, sb, aggs, w_t, ident_t, tnt, nblk,
                         out_dt=TDT)
    nc.compile()
    return nc


# --------------------------------------------------------------------------
# host-side graph preprocessing (integer/layout work only)
# --------------------------------------------------------------------------

def _preprocess(src, dst):
    src = src.astype(np.int64)
    dst = dst.astype(np.int64)
    E = src.shape[0]
    deg_out = np.bincount(src, minlength=N).astype(np.float32)
    deg_in = np.bincount(dst, minlength=N).astype(np.float32)

    core = dst // NPC
    bank = src // BANK_ROWS
    block = (dst % NPC) // P           # 0..NB-1 within core
    group = block // GROUP_BLOCKS
    # cell id in stream order: (group, bank, block-within-group)
    cell = (group * BANKS + bank) * GROUP_BLOCKS + (block % GROUP_BLOCKS)
    ncells = NGROUPS * BANKS * GROUP_BLOCKS

    order = np.lexsort((cell, core))
    core_s = core[order]
    cell_s = cell[order]
    src_s = src[order]
    dst_s = dst[order]

    counts = np.zeros((N_CORES, ncells), np.int64)
    flat = core * ncells + cell
    cnt = np.bincount(flat, minlength=N_CORES * ncells)
    counts = cnt.reshape(N_CORES, ncells)

    tiles = (counts + P - 1) // P
    tiles_u = tiles.max(axis=0)         # uniform per-cell tile count
    # guarantee every dst block has >=1 tile in its first present bank: give
    # bank 0's cell a minimum of one tile when the whole block is empty.
    tpc = np.zeros((BANKS, NB), np.int64)
    for c in range(ncells):
        g, b, dd = c // (BANKS * GROUP_BLOCKS), (c // GROUP_BLOCKS) % BANKS, c % GROUP_BLOCKS
        tpc[b][g * GROUP_BLOCKS + dd] = tiles_u[c]
    for d in range(NB):
        if tpc[:, d].sum() == 0:
            tpc[0][d] = 1
            g, dd = d // GROUP_BLOCKS, d % GROUP_BLOCKS
            tiles_u[(g * BANKS + 0) * GROUP_BLOCKS + dd] = 1

    slots = tiles_u * P
    cell_base = np.zeros(ncells + 1, np.int64)
    np.cumsum(slots, out=cell_base[1:])
    L = int(cell_base[-1])              # padded edge-slot count per core
    LT = L // P

    # position of each (sorted) edge inside its core's padded stream
    core_starts = np.searchsorted(core_s, np.arange(N_CORES + 1))
    idx_all = np.zeros((N_CORES, L), np.int64)       # bank-local src index
    dv_all = np.full((N_CORES, L), -1.0, np.float32)  # block-local dst or -1
    for c in range(N_CORES):
        lo, hi = core_starts[c], core_starts[c + 1]
        cells_c = cell_s[lo:hi]
        # rank within cell
        starts = np.searchsorted(cells_c, np.arange(ncells))
        rank = np.arange(hi - lo) - starts[cells_c]
        pos = cell_base[cells_c] + rank
        idx_all[c][pos] = src_s[lo:hi] % BANK_ROWS
        dv_all[c][pos] = (dst_s[lo:hi] % NPC) % P

    per_core = []
    for c in range(N_CORES):
        idx_wrapped = np.tile(
            idx_all[c].astype(np.int16).reshape(L // 16, 16).T, (8, 1))
        dv_arr = dv_all[c].reshape(LT, P).T.copy()
        per_core.append((idx_wrapped, dv_arr))

    deg_in_sh = [deg_in[c * NPC:(c + 1) * NPC].reshape(NB, P).T.copy()
                 for c in range(N_CORES)]
    deg_out_sh = [deg_out[c * NPC:(c + 1) * NPC].reshape(NB, P).T.copy()
                  for c in range(N_CORES)]
    return tpc.tolist(), per_core, deg_in_sh, deg_out_sh


# --------------------------------------------------------------------------
# entry point
# --------------------------------------------------------------------------

def kernel(feat, W1, b1, W2, b2, src, dst):
    global LAST_EXEC_NS
    LAST_EXEC_NS = []
    _install_ntff_shim()

    feat = np.asarray(feat, np.float32)
    W1 = np.asarray(W1, np.float32)
    W2 = np.asarray(W2, np.float32)
    b1 = np.asarray(b1, np.float32)
    b2 = np.asarray(b2, np.float32)
    src = np.asarray(src)
    dst = np.asarray(dst)

    t0 = time.time()
    tpc, per_core, deg_in_sh, deg_out_sh = _preprocess(src, dst)
    featp = np.zeros((N, D), np.float32)
    featp[:N_RAW] = feat
    ident = np.eye(P, dtype=np.float32)
    iota = np.tile(np.arange(P, dtype=np.float32), (P, 1))
    b1r = np.tile(b1, (P, 1))
    b2r = np.tile(b2, (P, 1))
    if TRACE:
        print(f"[kernel] preprocess {time.time()-t0:.1f}s", flush=True)

    key = tuple(tuple(r) for r in tpc)
    if "dense" not in _cache:
        t = time.time()
        _cache["dense"] = _build_dense()
        if TRACE:
            print(f"[kernel] dense build+compile {time.time()-t:.1f}s", flush=True)
    if ("layer", key) not in _cache:
        t = time.time()
        _cache[("layer", key)] = _build_layer(tpc)
        if TRACE:
            print(f"[kernel] layer build+compile {time.time()-t:.1f}s", flush=True)
    nc_dense = _cache["dense"]
    nc_layer = _cache[("layer", key)]

    run_kw = dict(core_ids=list(range(N_CORES)), trace=TRACE)

    # ---- stage A: t1 = (feat * norm_s) @ W1 ----
    in_maps = [{
        "feat": featp[c * NPC:(c + 1) * NPC],
        "deg_out": deg_out_sh[c],
        "w1": W1, "ident": ident,
    } for c in range(N_CORES)]
    res = bass_utils.run_bass_kernel_spmd(nc_dense, in_maps, **run_kw)
    LAST_EXEC_NS.append(("dense_t1", res.exec_time_ns))
    t1_full = np.concatenate(
        [res.results[c]["t1t"].T for c in range(N_CORES)], axis=0)

    # ---- stage B/C: the two GCN layers ----
    table = np.ascontiguousarray(t1_full)
    out_shards = None
    for li, (bias_r, wnext) in enumerate([(b1r, W2), (b2r, W2)]):
        in_maps = [{
            "table": table,
            "idx16": per_core[c][0], "dstval": per_core[c][1],
            "deg_in": deg_in_sh[c], "deg_out": deg_out_sh[c],
            "wn": wnext, "bias": bias_r, "iota_c": iota, "ident": ident,
        } for c in range(N_CORES)]
        res = bass_utils.run_bass_kernel_spmd(nc_layer, in_maps, **run_kw)
        LAST_EXEC_NS.append((f"layer{li + 1}", res.exec_time_ns))
        if li == 0:
            table = np.ascontiguousarray(np.concatenate(
                [res.results[c]["tnt"].T for c in range(N_CORES)], axis=0))
        else:
            out_shards = [res.results[c]["out1"] for c in range(N_CORES)]

    out = np.concatenate(out_shards, axis=0)[:N_RAW]
    if TRACE:
        print(f"[kernel] total wall {time.time()-t0:.1f}s  exec: {LAST_EXEC_NS}",
              flush=True)
    return out.astype(np.float32)



# revision 3
# speedup vs baseline: 1.6772x; 1.6772x over previous
"""DGL-style 2-layer GCN encoder on 8 Trainium2 NeuronCores.

Strategy (graph/data parallel, dst-sharded):
  - Nodes padded 100000 -> 102400 and split into 8 shards of 12800 (100 blocks
    of 128 nodes per core). Each core owns the aggregation for its dst shard.
  - Algebraic reformulation (linear ops commute):
        layer(h) = (segsum((h*norm_s)[src] -> dst) * norm_d) @ W + b
                 = segsum(((h*norm_s) @ W)[src] -> dst) * norm_d + b
    so the dense matmul runs over nodes (PE-friendly) and only the sparse
    segment-sum runs over edges.
  - Sparse segment-sum per core: edges sorted by (block-group, src-bank,
    dst-block); gathered 512B rows via dma_gather (int16 indices, 4 banks of
    25600 rows); scatter-sum via one-hot matmuls on PE (lhsT = one-hot built
    by DVE is_equal against an iota row, rhs = gathered rows), accumulated in
    PSUM per dst block, flushed with the norm_d scale fused on ACT.
  - No device collectives: three small NEFFs (dense t1; one GCN layer,
    invoked twice). The host only concatenates/transposes shards between
    NEFF invocations (no host FLOPs on the model math).
"""

import os
import sys
import time
import types
from contextlib import ExitStack

import numpy as np

sys.path.insert(0, "/opt/trn_rl_repo")

import concourse.bacc as bacc
import concourse.bass as bass
import concourse.tile as tile
from concourse import mybir
from concourse import bass_utils

F32 = mybir.dt.float32
F16 = mybir.dt.float16
I16 = mybir.dt.int16
SPARSE_FP32 = bool(int(os.environ.get("GCN_FP32", "0")))
TDT = F32 if SPARSE_FP32 else F16          # device dtype of the gathered table
TNP = np.float32 if SPARSE_FP32 else np.float16
AF = mybir.ActivationFunctionType
ALU = mybir.AluOpType

P = 128
D = 128
N_RAW = 100000
N_CORES = 8
NPC = 12800            # nodes per core
NB = NPC // P          # 100 dst blocks per core
N = NPC * N_CORES      # 102400 padded node count
BANKS = 4
BANK_ROWS = N // BANKS  # 25600 (< int16 max)
GROUP_BLOCKS = 4       # dst blocks per PSUM group (share one PSUM bank tile)
NGROUPS = NB // GROUP_BLOCKS
MAX_CALL_TILES = 24    # max tiles (128 idxs each) per dma_gather call

TRACE = bool(int(os.environ.get("GCN_TRACE", "0")))
LAST_EXEC_NS = []      # [(name, exec_time_ns)] of the most recent kernel() call

_cache = {}


def _install_ntff_shim():
    if "antenv.axon_hooks" in sys.modules:
        return
    try:
        from trn_agent_boot.trn_boot import _ntff_profile_via_ctypes

        hook = _ntff_profile_via_ctypes("/opt/axon/libaxon_pjrt.so")
        mod = types.ModuleType("antenv.axon_hooks")
        mod.get_axon_ntff_profile_hook = lambda: hook
        mod.set_axon_ntff_profile_hook = lambda h: None
        sys.modules["antenv.axon_hooks"] = mod
        bass_utils.upload_artifacts = lambda tmpdir: "local://" + tmpdir
    except Exception:
        pass


# --------------------------------------------------------------------------
# device programs
# --------------------------------------------------------------------------

def _emit_norm(nc, sb, deg_ap, nblk):
    """norm = clip(deg,1)^-0.5 with two Newton refinements; [P, nblk] tile."""
    deg = sb.tile([P, nblk], F32, tag="norm_deg")
    nc.sync.dma_start(deg[:], deg_ap[:])
    dcl = sb.tile([P, nblk], F32, tag="norm_dcl")
    nc.vector.tensor_scalar_max(dcl[:], deg[:], 1.0)
    s = sb.tile([P, nblk], F32, tag="norm_s")
    nc.scalar.activation(s[:], dcl[:], AF.Sqrt)
    r = sb.tile([P, nblk], F32, tag="norm_r0")
    nc.vector.reciprocal(r[:], s[:])
    for it in range(2):
        t0 = sb.tile([P, nblk], F32, tag=f"norm_t{it}a")
        nc.vector.tensor_mul(t0[:], r[:], r[:])
        t1 = sb.tile([P, nblk], F32, tag=f"norm_t{it}b")
        nc.vector.tensor_mul(t1[:], t0[:], dcl[:])
        t2 = sb.tile([P, nblk], F32, tag=f"norm_t{it}c")
        nc.vector.tensor_scalar(t2[:], t1[:], -0.5, 1.5, ALU.mult, ALU.add)
        r2 = sb.tile([P, nblk], F32, tag=f"norm_r{it + 1}")
        nc.vector.tensor_mul(r2[:], r[:], t2[:])
        r = r2
    return r


def _emit_dense_tail(nc, ctx, tc, sb, src_sb, w_t, ident_t, out_dram, nblk,
                     out_dt=F32):
    """out_dram[P, nblk*P] (feat-major) = W.T @ src_sb.T per node block.

    src_sb: SBUF [P, nblk, P] node-major (partition=node%128).
    """
    pt = ctx.enter_context(tc.tile_pool(name="tailpsum", bufs=4, space="PSUM"))
    pm = ctx.enter_context(tc.tile_pool(name="tailmm", bufs=2, space="PSUM"))
    st = ctx.enter_context(tc.tile_pool(name="tailsb", bufs=3))
    CH = 4  # blocks per dense matmul (N=512)
    for c in range(nblk // CH):
        rhs = st.tile([P, CH * P], F32, tag="rhs")
        for k in range(CH):
            d = c * CH + k
            tp = pt.tile([P, P], F32, space="PSUM")
            nc.tensor.transpose(out=tp[:], in_=src_sb[:, d, :], identity=ident_t[:])
            nc.vector.tensor_copy(rhs[:, k * P:(k + 1) * P], tp[:])
        mm = pm.tile([P, CH * P], F32, space="PSUM")
        nc.tensor.matmul(out=mm[:], lhsT=w_t[:], rhs=rhs[:], start=True, stop=True)
        stg = st.tile([P, CH * P], out_dt, tag="stg")
        nc.scalar.activation(stg[:], mm[:], AF.Copy)
        nc.sync.dma_start(out_dram[:, c * CH * P:(c + 1) * CH * P], stg[:])


def _build_dense(n_nodes=NPC):
    """NEFF-A: t1.T = ((feat*norm_s) @ W1).T for this core's shard."""
    nblk = n_nodes // P
    nc = bacc.Bacc("TRN2", target_bir_lowering=False, debug=False,
                   enable_asserts=False, num_devices=N_CORES)
    feat = nc.dram_tensor("feat", [n_nodes, D], F32, kind="ExternalInput").ap()
    deg_out = nc.dram_tensor("deg_out", [P, nblk], F32, kind="ExternalInput").ap()
    w1 = nc.dram_tensor("w1", [D, D], F32, kind="ExternalInput").ap()
    ident = nc.dram_tensor("ident", [P, P], F32, kind="ExternalInput").ap()
    t1t = nc.dram_tensor("t1t", [P, n_nodes], TDT, kind="ExternalOutput").ap()

    with tile.TileContext(nc) as tc, ExitStack() as ctx:
        sb = ctx.enter_context(tc.tile_pool(name="sb", bufs=1))
        ident_t = sb.tile([P, P], F32)
        nc.sync.dma_start(ident_t[:], ident[:])
        w_t = sb.tile([P, P], F32)
        nc.sync.dma_start(w_t[:], w1[:])
        norm_s = _emit_norm(nc, sb, deg_out, nblk)

        fsb = sb.tile([P, nblk, P], F32)
        nc.sync.dma_start(fsb[:], feat.rearrange("(d p) f -> p d f", p=P))
        fs = sb.tile([P, nblk, P], F32)
        nc.vector.tensor_mul(
            fs[:], fsb[:], norm_s[:].unsqueeze(2).to_broadcast([P, nblk, P])
        )
        _emit_dense_tail(nc, ctx, tc, sb, fs, w_t, ident_t, t1t, nblk,
                         out_dt=TDT)
    nc.compile()
    return nc


def _build_layer(tiles_per_cell, n_nodes_tab=N, nblk=NB, banks=BANKS,
                 bank_rows=BANK_ROWS, group_blocks=GROUP_BLOCKS):
    """NEFF-X: one GCN layer for this core's dst shard.

    tiles_per_cell: [banks][nblk] ints, identical for all cores.
    outputs:
      out1: [nblk*P, D] node-major  = segsum(table[src]) * norm_d + bias
      tnt:  [P, nblk*P] feat-major  = (relu(out1) * norm_s) @ Wn, transposed
    """
    ngroups = nblk // group_blocks
    tot_tiles = sum(sum(r) for r in tiles_per_cell)
    nc = bacc.Bacc("TRN2", target_bir_lowering=False, debug=False,
                   enable_asserts=False, num_devices=N_CORES,
                   num_swdge_queues=4)
    table = nc.dram_tensor("table", [n_nodes_tab, D], TDT, kind="ExternalInput").ap()
    idx16 = nc.dram_tensor("idx16", [P, tot_tiles * 8], I16, kind="ExternalInput").ap()
    dstval = nc.dram_tensor("dstval", [P, tot_tiles], F32, kind="ExternalInput").ap()
    deg_in = nc.dram_tensor("deg_in", [P, nblk], F32, kind="ExternalInput").ap()
    deg_out = nc.dram_tensor("deg_out", [P, nblk], F32, kind="ExternalInput").ap()
    wn = nc.dram_tensor("wn", [D, D], F32, kind="ExternalInput").ap()
    bias = nc.dram_tensor("bias", [P, D], F32, kind="ExternalInput").ap()
    iota_c = nc.dram_tensor("iota_c", [P, P], F32, kind="ExternalInput").ap()
    ident = nc.dram_tensor("ident", [P, P], F32, kind="ExternalInput").ap()
    out1 = nc.dram_tensor("out1", [nblk * P, D], F32, kind="ExternalOutput").ap()
    tnt = nc.dram_tensor("tnt", [P, nblk * P], TDT, kind="ExternalOutput").ap()

    with tile.TileContext(nc) as tc, ExitStack() as ctx:
        sb = ctx.enter_context(tc.tile_pool(name="sb", bufs=1))
        ident_t = sb.tile([P, P], F32)
        nc.sync.dma_start(ident_t[:], ident[:])
        iota_t = sb.tile([P, P], F32)
        nc.sync.dma_start(iota_t[:], iota_c[:])
        w_t = sb.tile([P, P], F32)
        nc.sync.dma_start(w_t[:], wn[:])
        bias_t = sb.tile([P, P], F32)
        nc.sync.dma_start(bias_t[:], bias[:])
        norm_d = _emit_norm(nc, sb, deg_in, nblk)
        norm_s = _emit_norm(nc, sb, deg_out, nblk)

        aggs = sb.tile([P, nblk, P], F32)   # norm_d-scaled aggregation
        out1_sb = sb.tile([P, nblk, P], F32)

        with ExitStack() as sctx:
            gp = sctx.enter_context(tc.tile_pool(name="gather", bufs=6))
            sp = sctx.enter_context(tc.tile_pool(name="onehot", bufs=6))
            ip = sctx.enter_context(tc.tile_pool(name="idxpool", bufs=8))
            pp = sctx.enter_context(
                tc.tile_pool(name="cellpsum", bufs=6, space="PSUM"))

            # one PSUM accumulation group per (group, bank-tile): start on the
            # very first matmul of the group, stop on the very last.  Each
            # block has >=1 tile so every 128-col slice gets written.
            group_mm_total = [
                sum(tiles_per_cell[b][d]
                    for b in range(banks)
                    for d in range(g * group_blocks, (g + 1) * group_blocks))
                for g in range(ngroups)
            ]

            tile_off = 0  # global tile cursor (== order of the edge stream)
            call_no = 0
            for g in range(ngroups):
                blocks = range(g * group_blocks, (g + 1) * group_blocks)
                # one PSUM bank tile holds all blocks of the group
                gps = pp.tile([P, group_blocks * P], F32, space="PSUM",
                              name="cellpsum", tag="cellpsum")
                psums = {d: gps[:, (d % group_blocks) * P:
                                (d % group_blocks + 1) * P] for d in blocks}
                mm_done = 0
                # per-block remaining-tile counters to time the flushes
                blk_left = {d: sum(tiles_per_cell[b][d] for b in range(banks))
                            for d in blocks}
                for b in range(banks):
                    # tiles of this (group, bank), split into <=MAX_CALL_TILES calls
                    gb_tiles = []  # (d, j, call_rel_pos) assembled per call
                    for d in blocks:
                        for j in range(tiles_per_cell[b][d]):
                            gb_tiles.append((d, j))
                    pos = 0
                    while pos < len(gb_tiles):
                        call = gb_tiles[pos:pos + MAX_CALL_TILES]
                        ct = len(call)
                        ni = ct * P
                        idx_t = ip.tile([P, ct * 8], I16, tag="idx")
                        nc.sync.dma_start(
                            idx_t[:], idx16[:, tile_off * 8:(tile_off + ct) * 8])
                        dv_t = ip.tile([P, ct], F32, tag="dv")
                        nc.scalar.dma_start(
                            dv_t[:], dstval[:, tile_off:tile_off + ct])
                        G = gp.tile([P, ct, P], TDT, tag="G")
                        nc.gpsimd.dma_gather(
                            G[:], table[b * bank_rows:(b + 1) * bank_rows, :],
                            idx_t[:], ni, ni, D, single_packet=False,
                            queue_num=call_no % 4)
                        call_no += 1
                        S = sp.tile([P, ct, P], TDT, tag="S")
                        nc.vector.tensor_tensor(
                            out=S[:],
                            in0=dv_t[:].unsqueeze(2).to_broadcast([P, ct, P]),
                            in1=iota_t[:].unsqueeze(1).to_broadcast([P, ct, P]),
                            op=ALU.is_equal)
                        for k, (d, j) in enumerate(call):
                            is_start = mm_done == 0
                            is_stop = mm_done == group_mm_total[g] - 1
                            nc.tensor.matmul(
                                out=psums[d][:], lhsT=S[:, k, :], rhs=G[:, k, :],
                                start=is_start, stop=is_stop)
                            mm_done += 1
                            blk_left[d] -= 1
                            if is_stop:
                                # group complete: flush every block's slice
                                # with the norm_d scale fused on ACT
                                for d2 in blocks:
                                    assert blk_left[d2] == 0, (g, d2)
                                    nc.scalar.activation(
                                        out=aggs[:, d2, :], in_=psums[d2][:],
                                        func=AF.Copy,
                                        scale=norm_d[:, d2:d2 + 1])
                        tile_off += ct
                        pos += ct

        # out1 = aggs + bias
        nc.vector.tensor_add(
            out1_sb[:], aggs[:],
            bias_t[:].unsqueeze(1).to_broadcast([P, nblk, P]))
        nc.sync.dma_start(out1.rearrange("(d p) f -> p d f", p=P), out1_sb[:])

        # hs = relu(out1 * norm_s)  (norm_s >= 0 so scale commutes with relu)
        for d in range(nblk):
            nc.scalar.activation(
                out=aggs[:, d, :], in_=out1_sb[:, d, :], func=AF.Relu,
                scale=norm_s[:, d:d + 1])
        _emit_dense_tail(nc, ctx, tc, sb, aggs, w_t, ident_t, tnt, nblk,
                         out_dt=TDT)
    nc.compile()
    return nc


# --------------------------------------------------------------------------
# host-side graph preprocessing (integer/layout work only)
# --------------------------------------------------------------------------

def _preprocess(src, dst):
    src = src.astype(np.int64)
    dst = dst.astype(np.int64)
    E = src.shape[0]
    deg_out = np.bincount(src, minlength=N).astype(np.float32)
    deg_in = np.bincount(dst, minlength=N).astype(np.float32)

    core = dst // NPC
    bank = src // BANK_ROWS
    block = (dst % NPC) // P           # 0..NB-1 within core
    group = block // GROUP_BLOCKS
    # cell id in stream order: (group, bank, block-within-group)
    cell = (group * BANKS + bank) * GROUP_BLOCKS + (block % GROUP_BLOCKS)
    ncells = NGROUPS * BANKS * GROUP_BLOCKS

    order = np.lexsort((cell, core))
    core_s = core[order]
    cell_s = cell[order]
    src_s = src[order]
    dst_s = dst[order]

    counts = np.zeros((N_CORES, ncells), np.int64)
    flat = core * ncells + cell
    cnt = np.bincount(flat, minlength=N_CORES * ncells)
    counts = cnt.reshape(N_CORES, ncells)

    tiles = (counts + P - 1) // P
    tiles_u = tiles.max(axis=0)         # uniform per-cell tile count
    # guarantee every dst block has >=1 tile in its first present bank: give
    # bank 0's cell a minimum of one tile when the whole block is empty.
    tpc = np.zeros((BANKS, NB), np.int64)
    for c in range(ncells):
        g, b, dd = c // (BANKS * GROUP_BLOCKS), (c // GROUP_BLOCKS) % BANKS, c % GROUP_BLOCKS
        tpc[b][g * GROUP_BLOCKS + dd] = tiles_u[c]
    for d in range(NB):
        if tpc[:, d].sum() == 0:
            tpc[0][d] = 1
            g, dd = d // GROUP_BLOCKS, d % GROUP_BLOCKS
            tiles_u[(g * BANKS + 0) * GROUP_BLOCKS + dd] = 1

    slots = tiles_u * P
    cell_base = np.zeros(ncells + 1, np.int64)
    np.cumsum(slots, out=cell_base[1:])
    L = int(cell_base[-1])              # padded edge-slot count per core
    LT = L // P

    # position of each (sorted) edge inside its core's padded stream
    core_starts = np.searchsorted(core_s, np.arange(N_CORES + 1))
    idx_all = np.zeros((N_CORES, L), np.int64)       # bank-local src index
    dv_all = np.full((N_CORES, L), -1.0, np.float32)  # block-local dst or -1
    for c in range(N_CORES):
        lo, hi = core_starts[c], core_starts[c + 1]
        cells_c = cell_s[lo:hi]
        # rank within cell
        starts = np.searchsorted(cells_c, np.arange(ncells))
        rank = np.arange(hi - lo) - starts[cells_c]
        pos = cell_base[cells_c] + rank
        idx_all[c][pos] = src_s[lo:hi] % BANK_ROWS
        dv_all[c][pos] = (dst_s[lo:hi] % NPC) % P

    per_core = []
    for c in range(N_CORES):
        idx_wrapped = np.tile(
            idx_all[c].astype(np.int16).reshape(L // 16, 16).T, (8, 1))
        dv_arr = dv_all[c].reshape(LT, P).T.copy()
        per_core.append((idx_wrapped, dv_arr))

    deg_in_sh = [deg_in[c * NPC:(c + 1) * NPC].reshape(NB, P).T.copy()
                 for c in range(N_CORES)]
    deg_out_sh = [deg_out[c * NPC:(c + 1) * NPC].reshape(NB, P).T.copy()
                  for c in range(N_CORES)]
    return tpc.tolist(), per_core, deg_in_sh, deg_out_sh


# --------------------------------------------------------------------------
# entry point
# --------------------------------------------------------------------------

def kernel(feat, W1, b1, W2, b2, src, dst):
    global LAST_EXEC_NS
    LAST_EXEC_NS = []
    _install_ntff_shim()

    feat = np.asarray(feat, np.float32)
    W1 = np.asarray(W1, np.float32)
    W2 = np.asarray(W2, np.float32)
    b1 = np.asarray(b1, np.float32)
    b2 = np.asarray(b2, np.float32)
    src = np.asarray(src)
    dst = np.asarray(dst)

    t0 = time.time()
    tpc, per_core, deg_in_sh, deg_out_sh = _preprocess(src, dst)
    featp = np.zeros((N, D), np.float32)
    featp[:N_RAW] = feat
    ident = np.eye(P, dtype=np.float32)
    iota = np.tile(np.arange(P, dtype=np.float32), (P, 1))
    b1r = np.tile(b1, (P, 1))
    b2r = np.tile(b2, (P, 1))
    if TRACE:
        print(f"[kernel] preprocess {time.time()-t0:.1f}s", flush=True)

    key = tuple(tuple(r) for r in tpc)
    if "dense" not in _cache:
        t = time.time()
        _cache["dense"] = _build_dense()
        if TRACE:
            print(f"[kernel] dense build+compile {time.time()-t:.1f}s", flush=True)
    if ("layer", key) not in _cache:
        t = time.time()
        _cache[("layer", key)] = _build_layer(tpc)
        if TRACE:
            print(f"[kernel] layer build+compile {time.time()-t:.1f}s", flush=True)
    nc_dense = _cache["dense"]
    nc_layer = _cache[("layer", key)]

    run_kw = dict(core_ids=list(range(N_CORES)), trace=TRACE)

    # ---- stage A: t1 = (feat * norm_s) @ W1 ----
    in_maps = [{
        "feat": featp[c * NPC:(c + 1) * NPC],
        "deg_out": deg_out_sh[c],
        "w1": W1, "ident": ident,
    } for c in range(N_CORES)]
    res = bass_utils.run_bass_kernel_spmd(nc_dense, in_maps, **run_kw)
    LAST_EXEC_NS.append(("dense_t1", res.exec_time_ns))
    t1_full = np.concatenate(
        [res.results[c]["t1t"].T for c in range(N_CORES)], axis=0)

    # ---- stage B/C: the two GCN layers ----
    table = np.ascontiguousarray(t1_full)
    out_shards = None
    for li, (bias_r, wnext) in enumerate([(b1r, W2), (b2r, W2)]):
        in_maps = [{
            "table": table,
            "idx16": per_core[c][0], "dstval": per_core[c][1],
            "deg_in": deg_in_sh[c], "deg_out": deg_out_sh[c],
            "wn": wnext, "bias": bias_r, "iota_c": iota, "ident": ident,
        } for c in range(N_CORES)]
        res = bass_utils.run_bass_kernel_spmd(nc_layer, in_maps, **run_kw)
        LAST_EXEC_NS.append((f"layer{li + 1}", res.exec_time_ns))
        if li == 0:
            table = np.ascontiguousarray(np.concatenate(
                [res.results[c]["tnt"].T for c in range(N_CORES)], axis=0))
        else:
            out_shards = [res.results[c]["out1"] for c in range(N_CORES)]

    out = np.concatenate(out_shards, axis=0)[:N_RAW]
    if TRACE:
        print(f"[kernel] total wall {time.time()-t0:.1f}s  exec: {LAST_EXEC_NS}",
              flush=True)
    return out.astype(np.float32)



# revision 11
# speedup vs baseline: 1.7061x; 1.0172x over previous
"""DGL-style 2-layer GCN encoder on 8 Trainium2 NeuronCores.

Strategy (graph/data parallel, dst-sharded):
  - Nodes padded 100000 -> 102400 and split into 8 shards of 12800 (100 blocks
    of 128 nodes per core). Each core owns the aggregation for its dst shard.
  - Algebraic reformulation (linear ops commute):
        layer(h) = (segsum((h*norm_s)[src] -> dst) * norm_d) @ W + b
                 = segsum(((h*norm_s) @ W)[src] -> dst) * norm_d + b
    so the dense matmul runs over nodes (PE-friendly) and only the sparse
    segment-sum runs over edges.
  - Sparse segment-sum per core: edges sorted by (block-group, src-bank,
    dst-block); gathered 512B rows via dma_gather (int16 indices, 4 banks of
    25600 rows); scatter-sum via one-hot matmuls on PE (lhsT = one-hot built
    by DVE is_equal against an iota row, rhs = gathered rows), accumulated in
    PSUM per dst block, flushed with the norm_d scale fused on ACT.
  - No device collectives: three small NEFFs (dense t1; one GCN layer,
    invoked twice). The host only concatenates/transposes shards between
    NEFF invocations (no host FLOPs on the model math).
"""

import os
import sys
import time
import types
from contextlib import ExitStack

import numpy as np

sys.path.insert(0, "/opt/trn_rl_repo")

import concourse.bacc as bacc
import concourse.bass as bass
import concourse.tile as tile
from concourse import mybir
from concourse import bass_utils

F32 = mybir.dt.float32
F16 = mybir.dt.float16
I16 = mybir.dt.int16
SPARSE_FP32 = bool(int(os.environ.get("GCN_FP32", "0")))
TDT = F32 if SPARSE_FP32 else F16          # device dtype of the gathered table
TNP = np.float32 if SPARSE_FP32 else np.float16
AF = mybir.ActivationFunctionType
ALU = mybir.AluOpType

P = 128
D = 128
N_RAW = 100000
N_CORES = 8
NPC = 12800            # nodes per core
NB = NPC // P          # 100 dst blocks per core
N = NPC * N_CORES      # 102400 padded node count
BANKS = 4
BANK_ROWS = N // BANKS  # 25600 (< int16 max)
GROUP_BLOCKS = 4       # dst blocks per PSUM group (share one PSUM bank tile)
NGROUPS = NB // GROUP_BLOCKS
MAX_CALL_TILES = 15    # max tiles per dma_gather call; 15*128=1920 rows keeps
                       # descs/dma-ring (=rows/16+1) under the 128-slot SWDGE
                       # ring so decode never stalls the sequencer on drain

TRACE = bool(int(os.environ.get("GCN_TRACE", "0")))
LAST_EXEC_NS = []      # [(name, exec_time_ns)] of the most recent kernel() call

_cache = {}


def _install_ntff_shim():
    if "antenv.axon_hooks" in sys.modules:
        return
    try:
        from trn_agent_boot.trn_boot import _ntff_profile_via_ctypes

        hook = _ntff_profile_via_ctypes("/opt/axon/libaxon_pjrt.so")
        mod = types.ModuleType("antenv.axon_hooks")
        mod.get_axon_ntff_profile_hook = lambda: hook
        mod.set_axon_ntff_profile_hook = lambda h: None
        sys.modules["antenv.axon_hooks"] = mod
        bass_utils.upload_artifacts = lambda tmpdir: "local://" + tmpdir
    except Exception:
        pass


# --------------------------------------------------------------------------
# device programs
# --------------------------------------------------------------------------

def _emit_norm(nc, sb, deg_ap, nblk):
    """norm = clip(deg,1)^-0.5 with two Newton refinements; [P, nblk] tile."""
    deg = sb.tile([P, nblk], F32, tag="norm_deg")
    nc.sync.dma_start(deg[:], deg_ap[:])
    dcl = sb.tile([P, nblk], F32, tag="norm_dcl")
    nc.vector.tensor_scalar_max(dcl[:], deg[:], 1.0)
    s = sb.tile([P, nblk], F32, tag="norm_s")
    nc.scalar.activation(s[:], dcl[:], AF.Sqrt)
    r = sb.tile([P, nblk], F32, tag="norm_r0")
    nc.vector.reciprocal(r[:], s[:])
    for it in range(2):
        t0 = sb.tile([P, nblk], F32, tag=f"norm_t{it}a")
        nc.vector.tensor_mul(t0[:], r[:], r[:])
        t1 = sb.tile([P, nblk], F32, tag=f"norm_t{it}b")
        nc.vector.tensor_mul(t1[:], t0[:], dcl[:])
        t2 = sb.tile([P, nblk], F32, tag=f"norm_t{it}c")
        nc.vector.tensor_scalar(t2[:], t1[:], -0.5, 1.5, ALU.mult, ALU.add)
        r2 = sb.tile([P, nblk], F32, tag=f"norm_r{it + 1}")
        nc.vector.tensor_mul(r2[:], r[:], t2[:])
        r = r2
    return r


def _emit_dense_tail(nc, ctx, tc, sb, src_sb, w_t, ident_t, out_dram, nblk,
                     out_dt=F32):
    """out_dram[P, nblk*P] (feat-major) = W.T @ src_sb.T per node block.

    src_sb: SBUF [P, nblk, P] node-major (partition=node%128).
    """
    pt = ctx.enter_context(tc.tile_pool(name="tailpsum", bufs=4, space="PSUM"))
    pm = ctx.enter_context(tc.tile_pool(name="tailmm", bufs=2, space="PSUM"))
    st = ctx.enter_context(tc.tile_pool(name="tailsb", bufs=3))
    CH = 4  # blocks per dense matmul (N=512)
    for c in range(nblk // CH):
        rhs = st.tile([P, CH * P], F32, tag="rhs")
        for k in range(CH):
            d = c * CH + k
            tp = pt.tile([P, P], F32, space="PSUM")
            nc.tensor.transpose(out=tp[:], in_=src_sb[:, d, :], identity=ident_t[:])
            nc.vector.tensor_copy(rhs[:, k * P:(k + 1) * P], tp[:])
        mm = pm.tile([P, CH * P], F32, space="PSUM")
        nc.tensor.matmul(out=mm[:], lhsT=w_t[:], rhs=rhs[:], start=True, stop=True)
        stg = st.tile([P, CH * P], out_dt, tag="stg")
        nc.scalar.activation(stg[:], mm[:], AF.Copy)
        nc.sync.dma_start(out_dram[:, c * CH * P:(c + 1) * CH * P], stg[:])


def _build_dense(n_nodes=NPC):
    """NEFF-A: t1.T = ((feat*norm_s) @ W1).T for this core's shard."""
    nblk = n_nodes // P
    nc = bacc.Bacc("TRN2", target_bir_lowering=False, debug=False,
                   enable_asserts=False, num_devices=N_CORES)
    feat = nc.dram_tensor("feat", [n_nodes, D], F32, kind="ExternalInput").ap()
    deg_out = nc.dram_tensor("deg_out", [P, nblk], F32, kind="ExternalInput").ap()
    w1 = nc.dram_tensor("w1", [D, D], F32, kind="ExternalInput").ap()
    ident = nc.dram_tensor("ident", [P, P], F32, kind="ExternalInput").ap()
    t1t = nc.dram_tensor("t1t", [P, n_nodes], TDT, kind="ExternalOutput").ap()

    with tile.TileContext(nc) as tc, ExitStack() as ctx:
        sb = ctx.enter_context(tc.tile_pool(name="sb", bufs=1))
        ident_t = sb.tile([P, P], F32)
        nc.sync.dma_start(ident_t[:], ident[:])
        w_t = sb.tile([P, P], F32)
        nc.sync.dma_start(w_t[:], w1[:])
        norm_s = _emit_norm(nc, sb, deg_out, nblk)

        fsb = sb.tile([P, nblk, P], F32)
        nc.sync.dma_start(fsb[:], feat.rearrange("(d p) f -> p d f", p=P))
        fs = sb.tile([P, nblk, P], F32)
        nc.vector.tensor_mul(
            fs[:], fsb[:], norm_s[:].unsqueeze(2).to_broadcast([P, nblk, P])
        )
        _emit_dense_tail(nc, ctx, tc, sb, fs, w_t, ident_t, t1t, nblk,
                         out_dt=TDT)
    nc.compile()
    return nc


def _build_layer(tiles_per_cell, n_nodes_tab=N, nblk=NB, banks=BANKS,
                 bank_rows=BANK_ROWS, group_blocks=GROUP_BLOCKS):
    """NEFF-X: one GCN layer for this core's dst shard.

    tiles_per_cell: [banks][nblk] ints, identical for all cores.
    outputs:
      out1: [nblk*P, D] node-major  = segsum(table[src]) * norm_d + bias
      tnt:  [P, nblk*P] feat-major  = (relu(out1) * norm_s) @ Wn, transposed
    """
    ngroups = nblk // group_blocks
    tot_tiles = sum(sum(r) for r in tiles_per_cell)
    nc = bacc.Bacc("TRN2", target_bir_lowering=False, debug=False,
                   enable_asserts=False, num_devices=N_CORES,
                   num_swdge_queues=4)
    table = nc.dram_tensor("table", [n_nodes_tab, D], TDT, kind="ExternalInput").ap()
    idx16 = nc.dram_tensor("idx16", [P, tot_tiles * 8], I16, kind="ExternalInput").ap()
    dstval = nc.dram_tensor("dstval", [P, tot_tiles], F32, kind="ExternalInput").ap()
    deg_in = nc.dram_tensor("deg_in", [P, nblk], F32, kind="ExternalInput").ap()
    deg_out = nc.dram_tensor("deg_out", [P, nblk], F32, kind="ExternalInput").ap()
    wn = nc.dram_tensor("wn", [D, D], F32, kind="ExternalInput").ap()
    bias = nc.dram_tensor("bias", [P, D], F32, kind="ExternalInput").ap()
    iota_c = nc.dram_tensor("iota_c", [P, P], F32, kind="ExternalInput").ap()
    ident = nc.dram_tensor("ident", [P, P], F32, kind="ExternalInput").ap()
    out1 = nc.dram_tensor("out1", [nblk * P, D], F32, kind="ExternalOutput").ap()
    tnt = nc.dram_tensor("tnt", [P, nblk * P], TDT, kind="ExternalOutput").ap()

    with tile.TileContext(nc) as tc, ExitStack() as ctx:
        sb = ctx.enter_context(tc.tile_pool(name="sb", bufs=1))
        ident_t = sb.tile([P, P], F32)
        nc.sync.dma_start(ident_t[:], ident[:])
        iota_t = sb.tile([P, P], F32)
        nc.sync.dma_start(iota_t[:], iota_c[:])
        w_t = sb.tile([P, P], F32)
        nc.sync.dma_start(w_t[:], wn[:])
        bias_t = sb.tile([P, P], F32)
        nc.sync.dma_start(bias_t[:], bias[:])
        norm_d = _emit_norm(nc, sb, deg_in, nblk)
        norm_s = _emit_norm(nc, sb, deg_out, nblk)

        aggs = sb.tile([P, nblk, P], F32)   # norm_d-scaled aggregation
        out1_sb = sb.tile([P, nblk, P], F32)

        with ExitStack() as sctx:
            gp = sctx.enter_context(tc.tile_pool(name="gather", bufs=6))
            sp = sctx.enter_context(tc.tile_pool(name="onehot", bufs=6))
            ip = sctx.enter_context(tc.tile_pool(name="idxpool", bufs=8))
            pp = sctx.enter_context(
                tc.tile_pool(name="cellpsum", bufs=6, space="PSUM"))

            # prime the G pool: trailing (idx=-1-trimmed) gather slots are
            # never written, so zero all rotating buffers once to avoid
            # NaN garbage flowing into the scatter matmuls.
            for _ in range(6):
                gz = gp.tile([P, MAX_CALL_TILES, P], TDT, tag="G")
                nc.vector.memset(gz[:], 0.0)

            # one PSUM accumulation group per (group, bank-tile): start on the
            # very first matmul of the group, stop on the very last.  Each
            # block has >=1 tile so every 128-col slice gets written.
            group_mm_total = [
                sum(tiles_per_cell[b][d]
                    for b in range(banks)
                    for d in range(g * group_blocks, (g + 1) * group_blocks))
                for g in range(ngroups)
            ]

            tile_off = 0  # global tile cursor (== order of the edge stream)
            call_no = 0
            for g in range(ngroups):
                blocks = range(g * group_blocks, (g + 1) * group_blocks)
                # one PSUM bank tile holds all blocks of the group
                gps = pp.tile([P, group_blocks * P], F32, space="PSUM",
                              name="cellpsum", tag="cellpsum")
                psums = {d: gps[:, (d % group_blocks) * P:
                                (d % group_blocks + 1) * P] for d in blocks}
                mm_done = 0
                # per-block remaining-tile counters to time the flushes
                blk_left = {d: sum(tiles_per_cell[b][d] for b in range(banks))
                            for d in blocks}
                for b in range(banks):
                    # tiles of this (group, bank), split into <=MAX_CALL_TILES calls
                    gb_tiles = []  # (d, j, call_rel_pos) assembled per call
                    for d in blocks:
                        for j in range(tiles_per_cell[b][d]):
                            gb_tiles.append((d, j))
                    # balanced call splits, each <= MAX_CALL_TILES tiles
                    T = len(gb_tiles)
                    nch = (T + MAX_CALL_TILES - 1) // MAX_CALL_TILES
                    sizes = []
                    if nch:
                        base, rem = divmod(T, nch)
                        sizes = [base + 1] * rem + [base] * (nch - rem)
                    pos = 0
                    for ct in sizes:
                        call = gb_tiles[pos:pos + ct]
                        ni = ct * P
                        idx_t = ip.tile([P, ct * 8], I16, tag="idx")
                        nc.sync.dma_start(
                            idx_t[:], idx16[:, tile_off * 8:(tile_off + ct) * 8])
                        dv_t = ip.tile([P, ct], F32, tag="dv")
                        nc.scalar.dma_start(
                            dv_t[:], dstval[:, tile_off:tile_off + ct])
                        G = gp.tile([P, ct, P], TDT, tag="G")
                        nc.gpsimd.dma_gather(
                            G[:], table[b * bank_rows:(b + 1) * bank_rows, :],
                            idx_t[:], ni, ni, D, single_packet=False,
                            queue_num=call_no % 4)
                        call_no += 1
                        S = sp.tile([P, ct, P], TDT, tag="S")
                        nc.vector.tensor_tensor(
                            out=S[:],
                            in0=dv_t[:].unsqueeze(2).to_broadcast([P, ct, P]),
                            in1=iota_t[:].unsqueeze(1).to_broadcast([P, ct, P]),
                            op=ALU.is_equal)
                        for k, (d, j) in enumerate(call):
                            is_start = mm_done == 0
                            is_stop = mm_done == group_mm_total[g] - 1
                            nc.tensor.matmul(
                                out=psums[d][:], lhsT=S[:, k, :], rhs=G[:, k, :],
                                start=is_start, stop=is_stop)
                            mm_done += 1
                            blk_left[d] -= 1
                            if is_stop:
                                # group complete: flush every block's slice
                                # with the norm_d scale fused on ACT
                                for d2 in blocks:
                                    assert blk_left[d2] == 0, (g, d2)
                                    nc.scalar.activation(
                                        out=aggs[:, d2, :], in_=psums[d2][:],
                                        func=AF.Copy,
                                        scale=norm_d[:, d2:d2 + 1])
                        tile_off += ct
                        pos += ct

        # out1 = aggs + bias
        nc.vector.tensor_add(
            out1_sb[:], aggs[:],
            bias_t[:].unsqueeze(1).to_broadcast([P, nblk, P]))
        nc.sync.dma_start(out1.rearrange("(d p) f -> p d f", p=P), out1_sb[:])

        # hs = relu(out1 * norm_s)  (norm_s >= 0 so scale commutes with relu)
        for d in range(nblk):
            nc.scalar.activation(
                out=aggs[:, d, :], in_=out1_sb[:, d, :], func=AF.Relu,
                scale=norm_s[:, d:d + 1])
        _emit_dense_tail(nc, ctx, tc, sb, aggs, w_t, ident_t, tnt, nblk,
                         out_dt=TDT)
    nc.compile()
    return nc


# --------------------------------------------------------------------------
# host-side graph preprocessing (integer/layout work only)
# --------------------------------------------------------------------------

def _preprocess(src, dst):
    src = src.astype(np.int64)
    dst = dst.astype(np.int64)
    E = src.shape[0]
    deg_out = np.bincount(src, minlength=N).astype(np.float32)
    deg_in = np.bincount(dst, minlength=N).astype(np.float32)

    core = dst // NPC
    bank = src // BANK_ROWS
    block = (dst % NPC) // P           # 0..NB-1 within core
    group = block // GROUP_BLOCKS
    # cell id in stream order: (group, bank, block-within-group)
    cell = (group * BANKS + bank) * GROUP_BLOCKS + (block % GROUP_BLOCKS)
    ncells = NGROUPS * BANKS * GROUP_BLOCKS

    # sort by (core, cell, src): src-ascending within a cell gives the
    # random-row gather some HBM locality, and is required for nothing else
    order = np.lexsort((src, cell, core))
    core_s = core[order]
    cell_s = cell[order]
    src_s = src[order]
    dst_s = dst[order]

    counts = np.zeros((N_CORES, ncells), np.int64)
    flat = core * ncells + cell
    cnt = np.bincount(flat, minlength=N_CORES * ncells)
    counts = cnt.reshape(N_CORES, ncells)

    tiles = (counts + P - 1) // P
    tiles_u = tiles.max(axis=0)         # uniform per-cell tile count
    # guarantee every dst block has >=1 tile in its first present bank: give
    # bank 0's cell a minimum of one tile when the whole block is empty.
    tpc = np.zeros((BANKS, NB), np.int64)
    for c in range(ncells):
        g, b, dd = c // (BANKS * GROUP_BLOCKS), (c // GROUP_BLOCKS) % BANKS, c % GROUP_BLOCKS
        tpc[b][g * GROUP_BLOCKS + dd] = tiles_u[c]
    for d in range(NB):
        if tpc[:, d].sum() == 0:
            tpc[0][d] = 1
            g, dd = d // GROUP_BLOCKS, d % GROUP_BLOCKS
            tiles_u[(g * BANKS + 0) * GROUP_BLOCKS + dd] = 1

    slots = tiles_u * P
    cell_base = np.zeros(ncells + 1, np.int64)
    np.cumsum(slots, out=cell_base[1:])
    L = int(cell_base[-1])              # padded edge-slot count per core
    LT = L // P

    # position of each (sorted) edge inside its core's padded stream
    core_starts = np.searchsorted(core_s, np.arange(N_CORES + 1))
    # padding slots start at idx=-1; all but the call-trailing ones are
    # rewritten to 0 below (the gather ucode only trims TRAILING negatives;
    # a mid-stream negative would be fetched at base-256, out of bounds)
    idx_all = np.full((N_CORES, L), -1, np.int64)    # bank-local src index
    dv_all = np.full((N_CORES, L), -1.0, np.float32)  # block-local dst or -1
    for c in range(N_CORES):
        lo, hi = core_starts[c], core_starts[c + 1]
        cells_c = cell_s[lo:hi]
        # rank within cell
        starts = np.searchsorted(cells_c, np.arange(ncells))
        rank = np.arange(hi - lo) - starts[cells_c]
        pos = cell_base[cells_c] + rank
        idx_all[c][pos] = src_s[lo:hi] % BANK_ROWS
        dv_all[c][pos] = (dst_s[lo:hi] % NPC) % P

    # every gather call ends at a (group, bank) boundary, i.e. at the end of
    # the group's LAST block-cell; only those cells' pads are call-trailing
    # and may stay -1 (trimmed for free).  All other pads become idx 0.
    cell_of_slot = np.searchsorted(cell_base[1:], np.arange(L), side="right")
    nonfinal = (cell_of_slot % GROUP_BLOCKS) != GROUP_BLOCKS - 1
    for c in range(N_CORES):
        idx_all[c][(idx_all[c] < 0)] = 0  # BISECT: trim disabled

    per_core = []
    for c in range(N_CORES):
        idx_wrapped = np.tile(
            idx_all[c].astype(np.int16).reshape(L // 16, 16).T, (8, 1))
        dv_arr = dv_all[c].reshape(LT, P).T.copy()
        per_core.append((idx_wrapped, dv_arr))

    deg_in_sh = [deg_in[c * NPC:(c + 1) * NPC].reshape(NB, P).T.copy()
                 for c in range(N_CORES)]
    deg_out_sh = [deg_out[c * NPC:(c + 1) * NPC].reshape(NB, P).T.copy()
                  for c in range(N_CORES)]
    return tpc.tolist(), per_core, deg_in_sh, deg_out_sh


# --------------------------------------------------------------------------
# entry point
# --------------------------------------------------------------------------

def kernel(feat, W1, b1, W2, b2, src, dst):
    global LAST_EXEC_NS
    LAST_EXEC_NS = []
    _install_ntff_shim()

    feat = np.asarray(feat, np.float32)
    W1 = np.asarray(W1, np.float32)
    W2 = np.asarray(W2, np.float32)
    b1 = np.asarray(b1, np.float32)
    b2 = np.asarray(b2, np.float32)
    src = np.asarray(src)
    dst = np.asarray(dst)

    t0 = time.time()
    tpc, per_core, deg_in_sh, deg_out_sh = _preprocess(src, dst)
    featp = np.zeros((N, D), np.float32)
    featp[:N_RAW] = feat
    ident = np.eye(P, dtype=np.float32)
    iota = np.tile(np.arange(P, dtype=np.float32), (P, 1))
    b1r = np.tile(b1, (P, 1))
    b2r = np.tile(b2, (P, 1))
    if TRACE:
        print(f"[kernel] preprocess {time.time()-t0:.1f}s", flush=True)

    key = tuple(tuple(r) for r in tpc)
    if "dense" not in _cache:
        t = time.time()
        _cache["dense"] = _build_dense()
        if TRACE:
            print(f"[kernel] dense build+compile {time.time()-t:.1f}s", flush=True)
    if ("layer", key) not in _cache:
        t = time.time()
        _cache[("layer", key)] = _build_layer(tpc)
        if TRACE:
            print(f"[kernel] layer build+compile {time.time()-t:.1f}s", flush=True)
    nc_dense = _cache["dense"]
    nc_layer = _cache[("layer", key)]

    run_kw = dict(core_ids=list(range(N_CORES)), trace=TRACE)

    # ---- stage A: t1 = (feat * norm_s) @ W1 ----
    in_maps = [{
        "feat": featp[c * NPC:(c + 1) * NPC],
        "deg_out": deg_out_sh[c],
        "w1": W1, "ident": ident,
    } for c in range(N_CORES)]
    res = bass_utils.run_bass_kernel_spmd(nc_dense, in_maps, **run_kw)
    LAST_EXEC_NS.append(("dense_t1", res.exec_time_ns))
    t1_full = np.concatenate(
        [res.results[c]["t1t"].T for c in range(N_CORES)], axis=0)

    # ---- stage B/C: the two GCN layers ----
    table = np.ascontiguousarray(t1_full)
    out_shards = None
    for li, (bias_r, wnext) in enumerate([(b1r, W2), (b2r, W2)]):
        in_maps = [{
            "table": table,
            "idx16": per_core[c][0], "dstval": per_core[c][1],
            "deg_in": deg_in_sh[c], "deg_out": deg_out_sh[c],
            "wn": wnext, "bias": bias_r, "iota_c": iota, "ident": ident,
        } for c in range(N_CORES)]
        res = bass_utils.run_bass_kernel_spmd(nc_layer, in_maps, **run_kw)
        LAST_EXEC_NS.append((f"layer{li + 1}", res.exec_time_ns))
        if li == 0:
            table = np.ascontiguousarray(np.concatenate(
                [res.results[c]["tnt"].T for c in range(N_CORES)], axis=0))
        else:
            out_shards = [res.results[c]["out1"] for c in range(N_CORES)]

    out = np.concatenate(out_shards, axis=0)[:N_RAW]
    if TRACE:
        print(f"[kernel] total wall {time.time()-t0:.1f}s  exec: {LAST_EXEC_NS}",
              flush=True)
    return out.astype(np.float32)



# revision 18
# speedup vs baseline: 1.7366x; 1.0179x over previous
"""DGL-style 2-layer GCN encoder on 8 Trainium2 NeuronCores.

Strategy (graph/data parallel, dst-sharded):
  - Nodes padded 100000 -> 102400 and split into 8 shards of 12800 (100 blocks
    of 128 nodes per core). Each core owns the aggregation for its dst shard.
  - Algebraic reformulation (linear ops commute):
        layer(h) = (segsum((h*norm_s)[src] -> dst) * norm_d) @ W + b
                 = segsum(((h*norm_s) @ W)[src] -> dst) * norm_d + b
    so the dense matmul runs over nodes (PE-friendly) and only the sparse
    segment-sum runs over edges.
  - Sparse segment-sum per core: edges sorted by (block-group, src-bank,
    dst-block); gathered 512B rows via dma_gather (int16 indices, 4 banks of
    25600 rows); scatter-sum via one-hot matmuls on PE (lhsT = one-hot built
    by DVE is_equal against an iota row, rhs = gathered rows), accumulated in
    PSUM per dst block, flushed with the norm_d scale fused on ACT.
  - No device collectives: three small NEFFs (dense t1; one GCN layer,
    invoked twice). The host only concatenates/transposes shards between
    NEFF invocations (no host FLOPs on the model math).
"""

import os
import sys
import time
import types
from contextlib import ExitStack

import numpy as np

sys.path.insert(0, "/opt/trn_rl_repo")

import concourse.bacc as bacc
import concourse.bass as bass
import concourse.tile as tile
from concourse import mybir
from concourse import bass_utils

F32 = mybir.dt.float32
F16 = mybir.dt.float16
F8 = mybir.dt.float8e4
I16 = mybir.dt.int16
SPARSE_FP32 = bool(int(os.environ.get("GCN_FP32", "0")))
TDT = F32 if SPARSE_FP32 else F16          # device dtype of the gathered table
TNP = np.float32 if SPARSE_FP32 else np.float16
AF = mybir.ActivationFunctionType
ALU = mybir.AluOpType

P = 128
D = 128
N_RAW = 100000
N_CORES = 8
NPC = 12800            # nodes per core
NB = NPC // P          # 100 dst blocks per core
N = NPC * N_CORES      # 102400 padded node count
BANKS = 4
BANK_ROWS = N // BANKS  # 25600 (< int16 max)
GROUP_BLOCKS = 4       # dst blocks per PSUM group (share one PSUM bank tile)
NGROUPS = NB // GROUP_BLOCKS
MAX_CALL_TILES = 15    # max tiles per dma_gather call; 15*128=1920 rows keeps
                       # descs/dma-ring (=rows/16+1) under the 128-slot SWDGE
                       # ring so decode never stalls the sequencer on drain

TRACE = bool(int(os.environ.get("GCN_TRACE", "0")))
LAST_EXEC_NS = []      # [(name, exec_time_ns)] of the most recent kernel() call

_cache = {}


def _install_ntff_shim():
    if "antenv.axon_hooks" in sys.modules:
        return
    try:
        from trn_agent_boot.trn_boot import _ntff_profile_via_ctypes

        hook = _ntff_profile_via_ctypes("/opt/axon/libaxon_pjrt.so")
        mod = types.ModuleType("antenv.axon_hooks")
        mod.get_axon_ntff_profile_hook = lambda: hook
        mod.set_axon_ntff_profile_hook = lambda h: None
        sys.modules["antenv.axon_hooks"] = mod
        bass_utils.upload_artifacts = lambda tmpdir: "local://" + tmpdir
    except Exception:
        pass


# --------------------------------------------------------------------------
# device programs
# --------------------------------------------------------------------------

def _emit_norm(nc, sb, deg_ap, nblk):
    """norm = clip(deg,1)^-0.5 with two Newton refinements; [P, nblk] tile."""
    deg = sb.tile([P, nblk], F32, tag="norm_deg")
    nc.sync.dma_start(deg[:], deg_ap[:])
    dcl = sb.tile([P, nblk], F32, tag="norm_dcl")
    nc.vector.tensor_scalar_max(dcl[:], deg[:], 1.0)
    s = sb.tile([P, nblk], F32, tag="norm_s")
    nc.scalar.activation(s[:], dcl[:], AF.Sqrt)
    r = sb.tile([P, nblk], F32, tag="norm_r0")
    nc.vector.reciprocal(r[:], s[:])
    for it in range(2):
        t0 = sb.tile([P, nblk], F32, tag=f"norm_t{it}a")
        nc.vector.tensor_mul(t0[:], r[:], r[:])
        t1 = sb.tile([P, nblk], F32, tag=f"norm_t{it}b")
        nc.vector.tensor_mul(t1[:], t0[:], dcl[:])
        t2 = sb.tile([P, nblk], F32, tag=f"norm_t{it}c")
        nc.vector.tensor_scalar(t2[:], t1[:], -0.5, 1.5, ALU.mult, ALU.add)
        r2 = sb.tile([P, nblk], F32, tag=f"norm_r{it + 1}")
        nc.vector.tensor_mul(r2[:], r[:], t2[:])
        r = r2
    return r


def _emit_dense_tail(nc, ctx, tc, sb, src_sb, w_t, ident_t, out_dram, nblk,
                     out_dt=F32):
    """out_dram[P, nblk*P] (feat-major) = W.T @ src_sb.T per node block.

    src_sb: SBUF [P, nblk, P] node-major (partition=node%128).
    """
    pt = ctx.enter_context(tc.tile_pool(name="tailpsum", bufs=4, space="PSUM"))
    pm = ctx.enter_context(tc.tile_pool(name="tailmm", bufs=2, space="PSUM"))
    st = ctx.enter_context(tc.tile_pool(name="tailsb", bufs=3))
    CH = 4  # blocks per dense matmul (N=512)
    for c in range(nblk // CH):
        rhs = st.tile([P, CH * P], F32, tag="rhs")
        for k in range(CH):
            d = c * CH + k
            tp = pt.tile([P, P], F32, space="PSUM")
            nc.tensor.transpose(out=tp[:], in_=src_sb[:, d, :], identity=ident_t[:])
            nc.vector.tensor_copy(rhs[:, k * P:(k + 1) * P], tp[:])
        mm = pm.tile([P, CH * P], F32, space="PSUM")
        nc.tensor.matmul(out=mm[:], lhsT=w_t[:], rhs=rhs[:], start=True, stop=True)
        stg = st.tile([P, CH * P], out_dt, tag="stg")
        nc.scalar.activation(stg[:], mm[:], AF.Copy)
        nc.sync.dma_start(out_dram[:, c * CH * P:(c + 1) * CH * P], stg[:])


def _build_dense(n_nodes=NPC):
    """NEFF-A: t1.T = ((feat*norm_s) @ W1).T for this core's shard."""
    nblk = n_nodes // P
    nc = bacc.Bacc("TRN2", target_bir_lowering=False, debug=False,
                   enable_asserts=False, num_devices=N_CORES)
    feat = nc.dram_tensor("feat", [n_nodes, D], F32, kind="ExternalInput").ap()
    deg_out = nc.dram_tensor("deg_out", [P, nblk], F32, kind="ExternalInput").ap()
    w1 = nc.dram_tensor("w1", [D, D], F32, kind="ExternalInput").ap()
    ident = nc.dram_tensor("ident", [P, P], F32, kind="ExternalInput").ap()
    t1t = nc.dram_tensor("t1t", [P, n_nodes], TDT, kind="ExternalOutput").ap()

    with tile.TileContext(nc) as tc, ExitStack() as ctx:
        sb = ctx.enter_context(tc.tile_pool(name="sb", bufs=1))
        ident_t = sb.tile([P, P], F32)
        nc.sync.dma_start(ident_t[:], ident[:])
        w_t = sb.tile([P, P], F32)
        nc.sync.dma_start(w_t[:], w1[:])
        norm_s = _emit_norm(nc, sb, deg_out, nblk)

        fsb = sb.tile([P, nblk, P], F32)
        nc.sync.dma_start(fsb[:], feat.rearrange("(d p) f -> p d f", p=P))
        fs = sb.tile([P, nblk, P], F32)
        nc.vector.tensor_mul(
            fs[:], fsb[:], norm_s[:].unsqueeze(2).to_broadcast([P, nblk, P])
        )
        _emit_dense_tail(nc, ctx, tc, sb, fs, w_t, ident_t, t1t, nblk,
                         out_dt=TDT)
    nc.compile()
    return nc


def _build_layer(tiles_per_cell, n_nodes_tab=N, nblk=NB, banks=BANKS,
                 bank_rows=BANK_ROWS, group_blocks=GROUP_BLOCKS):
    """NEFF-X: one GCN layer for this core's dst shard.

    tiles_per_cell: [banks][nblk] ints, identical for all cores.
    outputs:
      out1: [nblk*P, D] node-major  = segsum(table[src]) * norm_d + bias
      tnt:  [P, nblk*P] feat-major  = (relu(out1) * norm_s) @ Wn, transposed
    """
    ngroups = nblk // group_blocks
    tot_tiles = sum(sum(r) for r in tiles_per_cell)
    nc = bacc.Bacc("TRN2", target_bir_lowering=False, debug=False,
                   enable_asserts=False, num_devices=N_CORES,
                   num_swdge_queues=4)
    table = nc.dram_tensor("table", [n_nodes_tab, D], TDT, kind="ExternalInput").ap()
    idx16 = nc.dram_tensor("idx16", [P, tot_tiles * 8], I16, kind="ExternalInput").ap()
    # host-precomputed one-hot scatter matrices, fp8 (0/1 exact): S[e, t, s]
    smat = nc.dram_tensor("smat", [P, tot_tiles, P], F8, kind="ExternalInput").ap()
    deg_in = nc.dram_tensor("deg_in", [P, nblk], F32, kind="ExternalInput").ap()
    deg_out = nc.dram_tensor("deg_out", [P, nblk], F32, kind="ExternalInput").ap()
    wn = nc.dram_tensor("wn", [D, D], F32, kind="ExternalInput").ap()
    bias = nc.dram_tensor("bias", [P, D], F32, kind="ExternalInput").ap()
    ident = nc.dram_tensor("ident", [P, P], F32, kind="ExternalInput").ap()
    out1 = nc.dram_tensor("out1", [nblk * P, D], F32, kind="ExternalOutput").ap()
    tnt = nc.dram_tensor("tnt", [P, nblk * P], TDT, kind="ExternalOutput").ap()

    with tile.TileContext(nc) as tc, ExitStack() as ctx:
        sb = ctx.enter_context(tc.tile_pool(name="sb", bufs=1))
        ident_t = sb.tile([P, P], F32)
        nc.sync.dma_start(ident_t[:], ident[:])
        w_t = sb.tile([P, P], F32)
        nc.sync.dma_start(w_t[:], wn[:])
        bias_t = sb.tile([P, P], F32)
        nc.sync.dma_start(bias_t[:], bias[:])
        norm_d = _emit_norm(nc, sb, deg_in, nblk)
        norm_s = _emit_norm(nc, sb, deg_out, nblk)

        aggs = sb.tile([P, nblk, P], F32)   # norm_d-scaled aggregation
        out1_sb = sb.tile([P, nblk, P], F32)

        with ExitStack() as sctx:
            gp = sctx.enter_context(tc.tile_pool(name="gather", bufs=6))
            sp = sctx.enter_context(tc.tile_pool(name="onehot", bufs=6))
            ip = sctx.enter_context(tc.tile_pool(name="idxpool", bufs=8))
            pp = sctx.enter_context(
                tc.tile_pool(name="cellpsum", bufs=6, space="PSUM"))

            # prime the G pool: trailing (idx=-1-trimmed) gather slots are
            # never written, so zero all rotating buffers once to avoid
            # NaN garbage flowing into the scatter matmuls.
            for _ in range(6):
                gz = gp.tile([P, MAX_CALL_TILES, P], TDT, tag="G")
                nc.vector.memset(gz[:], 0.0)

            # one PSUM accumulation group per (group, bank-tile): start on the
            # very first matmul of the group, stop on the very last.  Each
            # block has >=1 tile so every 128-col slice gets written.
            group_mm_total = [
                sum(tiles_per_cell[b][d]
                    for b in range(banks)
                    for d in range(g * group_blocks, (g + 1) * group_blocks))
                for g in range(ngroups)
            ]

            tile_off = 0  # global tile cursor (== order of the edge stream)
            call_no = 0
            for g in range(ngroups):
                blocks = range(g * group_blocks, (g + 1) * group_blocks)
                # one PSUM bank tile holds all blocks of the group
                gps = pp.tile([P, group_blocks * P], F32, space="PSUM",
                              name="cellpsum", tag="cellpsum")
                psums = {d: gps[:, (d % group_blocks) * P:
                                (d % group_blocks + 1) * P] for d in blocks}
                mm_done = 0
                # per-block remaining-tile counters to time the flushes
                blk_left = {d: sum(tiles_per_cell[b][d] for b in range(banks))
                            for d in blocks}
                for b in range(banks):
                    # tiles of this (group, bank), split into <=MAX_CALL_TILES calls
                    gb_tiles = []  # (d, j, call_rel_pos) assembled per call
                    for d in blocks:
                        for j in range(tiles_per_cell[b][d]):
                            gb_tiles.append((d, j))
                    # balanced call splits, each <= MAX_CALL_TILES tiles
                    T = len(gb_tiles)
                    nch = (T + MAX_CALL_TILES - 1) // MAX_CALL_TILES
                    sizes = []
                    if nch:
                        base, rem = divmod(T, nch)
                        sizes = [base + 1] * rem + [base] * (nch - rem)
                    pos = 0
                    for ct in sizes:
                        call = gb_tiles[pos:pos + ct]
                        ni = ct * P
                        idx_t = ip.tile([P, ct * 8], I16, tag="idx")
                        nc.sync.dma_start(
                            idx_t[:], idx16[:, tile_off * 8:(tile_off + ct) * 8])
                        G = gp.tile([P, ct, P], TDT, tag="G")
                        nc.gpsimd.dma_gather(
                            G[:], table[b * bank_rows:(b + 1) * bank_rows, :],
                            idx_t[:], ni, ni, D, single_packet=False,
                            queue_num=call_no % 4)
                        call_no += 1
                        S = sp.tile([P, ct, P], F8, tag="S")
                        nc.scalar.dma_start(
                            S[:], smat[:, tile_off:tile_off + ct, :])
                        for k, (d, j) in enumerate(call):
                            is_start = mm_done == 0
                            is_stop = mm_done == group_mm_total[g] - 1
                            nc.tensor.matmul(
                                out=psums[d][:], lhsT=S[:, k, :], rhs=G[:, k, :],
                                start=is_start, stop=is_stop)
                            mm_done += 1
                            blk_left[d] -= 1
                            if is_stop:
                                # group complete: flush every block's slice
                                # with the norm_d scale fused on ACT
                                for d2 in blocks:
                                    assert blk_left[d2] == 0, (g, d2)
                                    nc.scalar.activation(
                                        out=aggs[:, d2, :], in_=psums[d2][:],
                                        func=AF.Copy,
                                        scale=norm_d[:, d2:d2 + 1])
                        tile_off += ct
                        pos += ct

        # out1 = aggs + bias
        nc.vector.tensor_add(
            out1_sb[:], aggs[:],
            bias_t[:].unsqueeze(1).to_broadcast([P, nblk, P]))
        nc.sync.dma_start(out1.rearrange("(d p) f -> p d f", p=P), out1_sb[:])

        # hs = relu(out1 * norm_s)  (norm_s >= 0 so scale commutes with relu)
        for d in range(nblk):
            nc.scalar.activation(
                out=aggs[:, d, :], in_=out1_sb[:, d, :], func=AF.Relu,
                scale=norm_s[:, d:d + 1])
        _emit_dense_tail(nc, ctx, tc, sb, aggs, w_t, ident_t, tnt, nblk,
                         out_dt=TDT)
    nc.compile()
    return nc


# --------------------------------------------------------------------------
# host-side graph preprocessing (integer/layout work only)
# --------------------------------------------------------------------------

def _preprocess(src, dst):
    src = src.astype(np.int64)
    dst = dst.astype(np.int64)
    E = src.shape[0]
    deg_out = np.bincount(src, minlength=N).astype(np.float32)
    deg_in = np.bincount(dst, minlength=N).astype(np.float32)

    core = dst // NPC
    bank = src // BANK_ROWS
    block = (dst % NPC) // P           # 0..NB-1 within core
    group = block // GROUP_BLOCKS
    # cell id in stream order: (group, bank, block-within-group)
    cell = (group * BANKS + bank) * GROUP_BLOCKS + (block % GROUP_BLOCKS)
    ncells = NGROUPS * BANKS * GROUP_BLOCKS

    # sort by (core, cell, src): src-ascending within a cell gives the
    # random-row gather some HBM locality, and is required for nothing else
    order = np.lexsort((src, cell, core))
    core_s = core[order]
    cell_s = cell[order]
    src_s = src[order]
    dst_s = dst[order]

    counts = np.zeros((N_CORES, ncells), np.int64)
    flat = core * ncells + cell
    cnt = np.bincount(flat, minlength=N_CORES * ncells)
    counts = cnt.reshape(N_CORES, ncells)

    tiles = (counts + P - 1) // P
    tiles_u = tiles.max(axis=0)         # uniform per-cell tile count
    # guarantee every dst block has >=1 tile in its first present bank: give
    # bank 0's cell a minimum of one tile when the whole block is empty.
    tpc = np.zeros((BANKS, NB), np.int64)
    for c in range(ncells):
        g, b, dd = c // (BANKS * GROUP_BLOCKS), (c // GROUP_BLOCKS) % BANKS, c % GROUP_BLOCKS
        tpc[b][g * GROUP_BLOCKS + dd] = tiles_u[c]
    for d in range(NB):
        if tpc[:, d].sum() == 0:
            tpc[0][d] = 1
            g, dd = d // GROUP_BLOCKS, d % GROUP_BLOCKS
            tiles_u[(g * BANKS + 0) * GROUP_BLOCKS + dd] = 1

    slots = tiles_u * P
    cell_base = np.zeros(ncells + 1, np.int64)
    np.cumsum(slots, out=cell_base[1:])
    L = int(cell_base[-1])              # padded edge-slot count per core
    LT = L // P

    # position of each (sorted) edge inside its core's padded stream
    core_starts = np.searchsorted(core_s, np.arange(N_CORES + 1))
    # padding slots start at idx=-1; all but the call-trailing ones are
    # rewritten to 0 below (the gather ucode only trims TRAILING negatives;
    # a mid-stream negative would be fetched at base-256, out of bounds)
    idx_all = np.full((N_CORES, L), -1, np.int64)    # bank-local src index
    dv_all = np.full((N_CORES, L), -1.0, np.float32)  # block-local dst or -1
    for c in range(N_CORES):
        lo, hi = core_starts[c], core_starts[c + 1]
        cells_c = cell_s[lo:hi]
        # rank within cell
        starts = np.searchsorted(cells_c, np.arange(ncells))
        rank = np.arange(hi - lo) - starts[cells_c]
        pos = cell_base[cells_c] + rank
        idx_all[c][pos] = src_s[lo:hi] % BANK_ROWS
        dv_all[c][pos] = (dst_s[lo:hi] % NPC) % P

    # every gather call ends at a (group, bank) boundary, i.e. at the end of
    # the group's LAST block-cell; only those cells' pads are call-trailing
    # and may stay -1 (trimmed for free).  All other pads become idx 0.
    cell_of_slot = np.searchsorted(cell_base[1:], np.arange(L), side="right")
    nonfinal = (cell_of_slot % GROUP_BLOCKS) != GROUP_BLOCKS - 1
    for c in range(N_CORES):
        idx_all[c][(idx_all[c] < 0)] = 0  # BISECT: trim disabled

    f8np = mybir.dt.np(F8)
    per_core = []
    for c in range(N_CORES):
        idx_wrapped = np.tile(
            idx_all[c].astype(np.int16).reshape(L // 16, 16).T, (8, 1))
        dvw = dv_all[c].reshape(LT, P).T  # [P, LT]
        # one-hot scatter matrices in fp8 (1.0 = 0x38); pads (dv=-1) stay 0
        smat_u8 = np.zeros((P, LT, P), np.uint8)
        pp, tt = np.nonzero(dvw >= 0)
        smat_u8[pp, tt, dvw[pp, tt].astype(np.int64)] = 0x38
        per_core.append((idx_wrapped, smat_u8.view(f8np)))

    deg_in_sh = [deg_in[c * NPC:(c + 1) * NPC].reshape(NB, P).T.copy()
                 for c in range(N_CORES)]
    deg_out_sh = [deg_out[c * NPC:(c + 1) * NPC].reshape(NB, P).T.copy()
                  for c in range(N_CORES)]
    return tpc.tolist(), per_core, deg_in_sh, deg_out_sh


# --------------------------------------------------------------------------
# entry point
# --------------------------------------------------------------------------

def kernel(feat, W1, b1, W2, b2, src, dst):
    global LAST_EXEC_NS
    LAST_EXEC_NS = []
    _install_ntff_shim()

    feat = np.asarray(feat, np.float32)
    W1 = np.asarray(W1, np.float32)
    W2 = np.asarray(W2, np.float32)
    b1 = np.asarray(b1, np.float32)
    b2 = np.asarray(b2, np.float32)
    src = np.asarray(src)
    dst = np.asarray(dst)

    t0 = time.time()
    tpc, per_core, deg_in_sh, deg_out_sh = _preprocess(src, dst)
    featp = np.zeros((N, D), np.float32)
    featp[:N_RAW] = feat
    ident = np.eye(P, dtype=np.float32)
    b1r = np.tile(b1, (P, 1))
    b2r = np.tile(b2, (P, 1))
    if TRACE:
        print(f"[kernel] preprocess {time.time()-t0:.1f}s", flush=True)

    key = tuple(tuple(r) for r in tpc)
    if "dense" not in _cache:
        t = time.time()
        _cache["dense"] = _build_dense()
        if TRACE:
            print(f"[kernel] dense build+compile {time.time()-t:.1f}s", flush=True)
    if ("layer", key) not in _cache:
        t = time.time()
        _cache[("layer", key)] = _build_layer(tpc)
        if TRACE:
            print(f"[kernel] layer build+compile {time.time()-t:.1f}s", flush=True)
    nc_dense = _cache["dense"]
    nc_layer = _cache[("layer", key)]

    run_kw = dict(core_ids=list(range(N_CORES)), trace=TRACE)

    # ---- stage A: t1 = (feat * norm_s) @ W1 ----
    in_maps = [{
        "feat": featp[c * NPC:(c + 1) * NPC],
        "deg_out": deg_out_sh[c],
        "w1": W1, "ident": ident,
    } for c in range(N_CORES)]
    res = bass_utils.run_bass_kernel_spmd(nc_dense, in_maps, **run_kw)
    LAST_EXEC_NS.append(("dense_t1", res.exec_time_ns))
    t1_full = np.concatenate(
        [res.results[c]["t1t"].T for c in range(N_CORES)], axis=0)

    # ---- stage B/C: the two GCN layers ----
    table = np.ascontiguousarray(t1_full)
    out_shards = None
    for li, (bias_r, wnext) in enumerate([(b1r, W2), (b2r, W2)]):
        in_maps = [{
            "table": table,
            "idx16": per_core[c][0], "smat": per_core[c][1],
            "deg_in": deg_in_sh[c], "deg_out": deg_out_sh[c],
            "wn": wnext, "bias": bias_r, "ident": ident,
        } for c in range(N_CORES)]
        res = bass_utils.run_bass_kernel_spmd(nc_layer, in_maps, **run_kw)
        LAST_EXEC_NS.append((f"layer{li + 1}", res.exec_time_ns))
        if li == 0:
            table = np.ascontiguousarray(np.concatenate(
                [res.results[c]["tnt"].T for c in range(N_CORES)], axis=0))
        else:
            out_shards = [res.results[c]["out1"] for c in range(N_CORES)]

    out = np.concatenate(out_shards, axis=0)[:N_RAW]
    if TRACE:
        print(f"[kernel] total wall {time.time()-t0:.1f}s  exec: {LAST_EXEC_NS}",
              flush=True)
    return out.astype(np.float32)



# revision 22
# speedup vs baseline: 1.7755x; 1.0224x over previous
"""DGL-style 2-layer GCN encoder on 8 Trainium2 NeuronCores.

Strategy (graph/data parallel, dst-sharded):
  - Nodes padded 100000 -> 102400 and split into 8 shards of 12800 (100 blocks
    of 128 nodes per core). Each core owns the aggregation for its dst shard.
  - Algebraic reformulation (linear ops commute):
        layer(h) = (segsum((h*norm_s)[src] -> dst) * norm_d) @ W + b
                 = segsum(((h*norm_s) @ W)[src] -> dst) * norm_d + b
    so the dense matmul runs over nodes (PE-friendly) and only the sparse
    segment-sum runs over edges.
  - Sparse segment-sum per core: edges sorted by (block-group, src-bank,
    dst-block); gathered 512B rows via dma_gather (int16 indices, 4 banks of
    25600 rows); scatter-sum via one-hot matmuls on PE (lhsT = one-hot built
    by DVE is_equal against an iota row, rhs = gathered rows), accumulated in
    PSUM per dst block, flushed with the norm_d scale fused on ACT.
  - No device collectives: three small NEFFs (dense t1; one GCN layer,
    invoked twice). The host only concatenates/transposes shards between
    NEFF invocations (no host FLOPs on the model math).
"""

import os
import sys
import time
import types
from contextlib import ExitStack

import numpy as np

sys.path.insert(0, "/opt/trn_rl_repo")

import concourse.bacc as bacc
import concourse.bass as bass
import concourse.tile as tile
from concourse import mybir
from concourse import bass_utils

F32 = mybir.dt.float32
F16 = mybir.dt.float16
F8 = mybir.dt.float8e4
I16 = mybir.dt.int16
SPARSE_FP32 = bool(int(os.environ.get("GCN_FP32", "0")))
TDT = F32 if SPARSE_FP32 else F16          # device dtype of the gathered table
TNP = np.float32 if SPARSE_FP32 else np.float16
AF = mybir.ActivationFunctionType
ALU = mybir.AluOpType

P = 128
D = 128
N_RAW = 100000
N_CORES = 8
NPC = 12800            # nodes per core
NB = NPC // P          # 100 dst blocks per core
N = NPC * N_CORES      # 102400 padded node count
BANKS = 4
BANK_ROWS = N // BANKS  # 25600 (< int16 max)
GROUP_BLOCKS = 4       # dst blocks per PSUM group (share one PSUM bank tile)
NGROUPS = NB // GROUP_BLOCKS
MAX_CALL_TILES = 15    # max tiles per dma_gather call; 15*128=1920 rows keeps
                       # descs/dma-ring (=rows/16+1) under the 128-slot SWDGE
                       # ring so decode never stalls the sequencer on drain

TRACE = bool(int(os.environ.get("GCN_TRACE", "0")))
LAST_EXEC_NS = []      # [(name, exec_time_ns)] of the most recent kernel() call

_cache = {}


def _install_ntff_shim():
    if "antenv.axon_hooks" in sys.modules:
        return
    try:
        from trn_agent_boot.trn_boot import _ntff_profile_via_ctypes

        hook = _ntff_profile_via_ctypes("/opt/axon/libaxon_pjrt.so")
        mod = types.ModuleType("antenv.axon_hooks")
        mod.get_axon_ntff_profile_hook = lambda: hook
        mod.set_axon_ntff_profile_hook = lambda h: None
        sys.modules["antenv.axon_hooks"] = mod
        bass_utils.upload_artifacts = lambda tmpdir: "local://" + tmpdir
    except Exception:
        pass


# --------------------------------------------------------------------------
# device programs
# --------------------------------------------------------------------------

def _emit_norm(nc, sb, deg_ap, nblk):
    """norm = clip(deg,1)^-0.5 with two Newton refinements; [P, nblk] tile."""
    deg = sb.tile([P, nblk], F32, tag="norm_deg")
    nc.sync.dma_start(deg[:], deg_ap[:])
    dcl = sb.tile([P, nblk], F32, tag="norm_dcl")
    nc.vector.tensor_scalar_max(dcl[:], deg[:], 1.0)
    s = sb.tile([P, nblk], F32, tag="norm_s")
    nc.scalar.activation(s[:], dcl[:], AF.Sqrt)
    r = sb.tile([P, nblk], F32, tag="norm_r0")
    nc.vector.reciprocal(r[:], s[:])
    for it in range(2):
        t0 = sb.tile([P, nblk], F32, tag=f"norm_t{it}a")
        nc.vector.tensor_mul(t0[:], r[:], r[:])
        t1 = sb.tile([P, nblk], F32, tag=f"norm_t{it}b")
        nc.vector.tensor_mul(t1[:], t0[:], dcl[:])
        t2 = sb.tile([P, nblk], F32, tag=f"norm_t{it}c")
        nc.vector.tensor_scalar(t2[:], t1[:], -0.5, 1.5, ALU.mult, ALU.add)
        r2 = sb.tile([P, nblk], F32, tag=f"norm_r{it + 1}")
        nc.vector.tensor_mul(r2[:], r[:], t2[:])
        r = r2
    return r


def _emit_dense_tail(nc, ctx, tc, sb, src_sb, w_t, ident_t, out_dram, nblk,
                     out_dt=F32):
    """out_dram[P, nblk*P] (feat-major) = W.T @ src_sb.T per node block.

    src_sb: SBUF [P, nblk, P] node-major (partition=node%128).
    """
    pt = ctx.enter_context(tc.tile_pool(name="tailpsum", bufs=4, space="PSUM"))
    pm = ctx.enter_context(tc.tile_pool(name="tailmm", bufs=2, space="PSUM"))
    st = ctx.enter_context(tc.tile_pool(name="tailsb", bufs=3))
    CH = 4  # blocks per dense matmul (N=512)
    for c in range(nblk // CH):
        rhs = st.tile([P, CH * P], F32, tag="rhs")
        for k in range(CH):
            d = c * CH + k
            tp = pt.tile([P, P], F32, space="PSUM")
            nc.tensor.transpose(out=tp[:], in_=src_sb[:, d, :], identity=ident_t[:])
            nc.vector.tensor_copy(rhs[:, k * P:(k + 1) * P], tp[:])
        mm = pm.tile([P, CH * P], F32, space="PSUM")
        nc.tensor.matmul(out=mm[:], lhsT=w_t[:], rhs=rhs[:], start=True, stop=True)
        stg = st.tile([P, CH * P], out_dt, tag="stg")
        nc.scalar.activation(stg[:], mm[:], AF.Copy)
        nc.sync.dma_start(out_dram[:, c * CH * P:(c + 1) * CH * P], stg[:])


def _build_dense(n_nodes=NPC):
    """NEFF-A: t1.T = ((feat*norm_s) @ W1).T for this core's shard."""
    nblk = n_nodes // P
    nc = bacc.Bacc("TRN2", target_bir_lowering=False, debug=False,
                   enable_asserts=False, num_devices=N_CORES)
    feat = nc.dram_tensor("feat", [n_nodes, D], F32, kind="ExternalInput").ap()
    deg_out = nc.dram_tensor("deg_out", [P, nblk], F32, kind="ExternalInput").ap()
    w1 = nc.dram_tensor("w1", [D, D], F32, kind="ExternalInput").ap()
    ident = nc.dram_tensor("ident", [P, P], F32, kind="ExternalInput").ap()
    t1t = nc.dram_tensor("t1t", [P, n_nodes], TDT, kind="ExternalOutput").ap()

    with tile.TileContext(nc) as tc, ExitStack() as ctx:
        sb = ctx.enter_context(tc.tile_pool(name="sb", bufs=1))
        ident_t = sb.tile([P, P], F32)
        nc.sync.dma_start(ident_t[:], ident[:])
        w_t = sb.tile([P, P], F32)
        nc.sync.dma_start(w_t[:], w1[:])
        norm_s = _emit_norm(nc, sb, deg_out, nblk)

        fsb = sb.tile([P, nblk, P], F32)
        nc.sync.dma_start(fsb[:], feat.rearrange("(d p) f -> p d f", p=P))
        fs = sb.tile([P, nblk, P], F32)
        nc.vector.tensor_mul(
            fs[:], fsb[:], norm_s[:].unsqueeze(2).to_broadcast([P, nblk, P])
        )
        _emit_dense_tail(nc, ctx, tc, sb, fs, w_t, ident_t, t1t, nblk,
                         out_dt=TDT)
    nc.compile()
    return nc


def _build_layer(tiles_per_cell, n_nodes_tab=N, nblk=NB, banks=BANKS,
                 bank_rows=BANK_ROWS, group_blocks=GROUP_BLOCKS):
    """NEFF-X: one GCN layer for this core's dst shard.

    tiles_per_cell: [banks][nblk] ints, identical for all cores.
    outputs:
      out1: [nblk*P, D] node-major  = segsum(table[src]) * norm_d + bias
      tnt:  [P, nblk*P] feat-major  = (relu(out1) * norm_s) @ Wn, transposed
    """
    ngroups = nblk // group_blocks
    tot_tiles = sum(sum(r) for r in tiles_per_cell)
    nc = bacc.Bacc("TRN2", target_bir_lowering=False, debug=False,
                   enable_asserts=False, num_devices=N_CORES,
                   num_swdge_queues=4)
    table = nc.dram_tensor("table", [n_nodes_tab, D], TDT, kind="ExternalInput").ap()
    idx16 = nc.dram_tensor("idx16", [P, tot_tiles * 8], I16, kind="ExternalInput").ap()
    # host-precomputed one-hot scatter matrices, fp8 (0/1 exact): S[e, t, s]
    smat = nc.dram_tensor("smat", [P, tot_tiles, P], F8, kind="ExternalInput").ap()
    deg_in = nc.dram_tensor("deg_in", [P, nblk], F32, kind="ExternalInput").ap()
    deg_out = nc.dram_tensor("deg_out", [P, nblk], F32, kind="ExternalInput").ap()
    wn = nc.dram_tensor("wn", [D, D], F32, kind="ExternalInput").ap()
    bias = nc.dram_tensor("bias", [P, D], F32, kind="ExternalInput").ap()
    ident = nc.dram_tensor("ident", [P, P], F32, kind="ExternalInput").ap()
    out1 = nc.dram_tensor("out1", [nblk * P, D], F32, kind="ExternalOutput").ap()
    tnt = nc.dram_tensor("tnt", [P, nblk * P], TDT, kind="ExternalOutput").ap()

    with tile.TileContext(nc) as tc, ExitStack() as ctx:
        sb = ctx.enter_context(tc.tile_pool(name="sb", bufs=1))
        ident_t = sb.tile([P, P], F32)
        nc.sync.dma_start(ident_t[:], ident[:])
        w_t = sb.tile([P, P], F32)
        nc.sync.dma_start(w_t[:], wn[:])
        bias_t = sb.tile([P, P], F32)
        nc.sync.dma_start(bias_t[:], bias[:])
        norm_d = _emit_norm(nc, sb, deg_in, nblk)
        norm_s = _emit_norm(nc, sb, deg_out, nblk)

        aggs = sb.tile([P, nblk, P], F32)   # norm_d-scaled aggregation
        out1_sb = sb.tile([P, nblk, P], F32)
        out1_r = out1.rearrange("(d p) f -> p d f", p=P)

        with ExitStack() as sctx:
            gp = sctx.enter_context(tc.tile_pool(name="gather", bufs=6))
            sp = sctx.enter_context(tc.tile_pool(name="onehot", bufs=6))
            ip = sctx.enter_context(tc.tile_pool(name="idxpool", bufs=8))
            pp = sctx.enter_context(
                tc.tile_pool(name="cellpsum", bufs=2, space="PSUM"))
            # interleaved dense-tail pools (coexist with the gather pools so
            # each group's tail hides under later groups' gathers)
            tp2 = sctx.enter_context(
                tc.tile_pool(name="tailtp", bufs=4, space="PSUM"))
            pm = sctx.enter_context(
                tc.tile_pool(name="tailmm", bufs=2, space="PSUM"))
            st = sctx.enter_context(tc.tile_pool(name="tailsb", bufs=3))

            # prime the G pool: trailing (idx=-1-trimmed) gather slots are
            # never written, so zero all rotating buffers once to avoid
            # NaN garbage flowing into the scatter matmuls.
            for _ in range(6):
                gz = gp.tile([P, MAX_CALL_TILES, P], TDT, tag="G")
                nc.vector.memset(gz[:], 0.0)

            # one PSUM accumulation group per (group, bank-tile): start on the
            # very first matmul of the group, stop on the very last.  Each
            # block has >=1 tile so every 128-col slice gets written.
            group_mm_total = [
                sum(tiles_per_cell[b][d]
                    for b in range(banks)
                    for d in range(g * group_blocks, (g + 1) * group_blocks))
                for g in range(ngroups)
            ]

            tile_off = 0  # global tile cursor (== order of the edge stream)
            call_no = 0
            for g in range(ngroups):
                blocks = range(g * group_blocks, (g + 1) * group_blocks)
                # one PSUM bank tile holds all blocks of the group
                gps = pp.tile([P, group_blocks * P], F32, space="PSUM",
                              name="cellpsum", tag="cellpsum")
                psums = {d: gps[:, (d % group_blocks) * P:
                                (d % group_blocks + 1) * P] for d in blocks}
                mm_done = 0
                # per-block remaining-tile counters to time the flushes
                blk_left = {d: sum(tiles_per_cell[b][d] for b in range(banks))
                            for d in blocks}
                for b in range(banks):
                    # tiles of this (group, bank), split into <=MAX_CALL_TILES calls
                    gb_tiles = []  # (d, j, call_rel_pos) assembled per call
                    for d in blocks:
                        for j in range(tiles_per_cell[b][d]):
                            gb_tiles.append((d, j))
                    # balanced call splits, each <= MAX_CALL_TILES tiles
                    T = len(gb_tiles)
                    nch = (T + MAX_CALL_TILES - 1) // MAX_CALL_TILES
                    sizes = []
                    if nch:
                        base, rem = divmod(T, nch)
                        sizes = [base + 1] * rem + [base] * (nch - rem)
                    pos = 0
                    for ct in sizes:
                        call = gb_tiles[pos:pos + ct]
                        ni = ct * P
                        idx_t = ip.tile([P, ct * 8], I16, tag="idx")
                        nc.sync.dma_start(
                            idx_t[:], idx16[:, tile_off * 8:(tile_off + ct) * 8])
                        G = gp.tile([P, ct, P], TDT, tag="G")
                        nc.gpsimd.dma_gather(
                            G[:], table[b * bank_rows:(b + 1) * bank_rows, :],
                            idx_t[:], ni, ni, D, single_packet=False,
                            queue_num=call_no % 4)
                        call_no += 1
                        S = sp.tile([P, ct, P], F8, tag="S")
                        nc.scalar.dma_start(
                            S[:], smat[:, tile_off:tile_off + ct, :])
                        for k, (d, j) in enumerate(call):
                            is_start = mm_done == 0
                            is_stop = mm_done == group_mm_total[g] - 1
                            nc.tensor.matmul(
                                out=psums[d][:], lhsT=S[:, k, :], rhs=G[:, k, :],
                                start=is_start, stop=is_stop)
                            mm_done += 1
                            blk_left[d] -= 1
                            if is_stop:
                                # group complete: flush every block's slice
                                # with the norm_d scale fused on ACT
                                for d2 in blocks:
                                    assert blk_left[d2] == 0, (g, d2)
                                    nc.scalar.activation(
                                        out=aggs[:, d2, :], in_=psums[d2][:],
                                        func=AF.Copy,
                                        scale=norm_d[:, d2:d2 + 1])
                                # ---- interleaved tail for this group ----
                                g0, g1 = g * group_blocks, (g + 1) * group_blocks
                                nc.vector.tensor_add(
                                    out1_sb[:, g0:g1, :], aggs[:, g0:g1, :],
                                    bias_t[:].unsqueeze(1).to_broadcast(
                                        [P, group_blocks, P]))
                                nc.sync.dma_start(
                                    out1_r[:, g0:g1, :], out1_sb[:, g0:g1, :])
                                # hs = relu(out1 * norm_s) (norm_s >= 0)
                                for d2 in blocks:
                                    nc.scalar.activation(
                                        out=aggs[:, d2, :],
                                        in_=out1_sb[:, d2, :], func=AF.Relu,
                                        scale=norm_s[:, d2:d2 + 1])
                                rhs = st.tile([P, group_blocks * P], F32,
                                              tag="rhs")
                                for k2, d2 in enumerate(blocks):
                                    tpt = tp2.tile([P, P], F32,
                                                   space="PSUM", tag="tp")
                                    nc.tensor.transpose(
                                        out=tpt[:], in_=aggs[:, d2, :],
                                        identity=ident_t[:])
                                    nc.vector.tensor_copy(
                                        rhs[:, k2 * P:(k2 + 1) * P], tpt[:])
                                mm = pm.tile([P, group_blocks * P], F32,
                                             space="PSUM", tag="mm")
                                nc.tensor.matmul(out=mm[:], lhsT=w_t[:],
                                                 rhs=rhs[:], start=True,
                                                 stop=True)
                                stg = st.tile([P, group_blocks * P], TDT,
                                              tag="stg")
                                nc.scalar.activation(stg[:], mm[:], AF.Copy)
                                nc.sync.dma_start(
                                    tnt[:, g0 * P:g1 * P], stg[:])
                        tile_off += ct
                        pos += ct
    nc.compile()
    return nc


# --------------------------------------------------------------------------
# host-side graph preprocessing (integer/layout work only)
# --------------------------------------------------------------------------

def _preprocess(src, dst):
    src = src.astype(np.int64)
    dst = dst.astype(np.int64)
    E = src.shape[0]
    deg_out = np.bincount(src, minlength=N).astype(np.float32)
    deg_in = np.bincount(dst, minlength=N).astype(np.float32)

    core = dst // NPC
    bank = src // BANK_ROWS
    block = (dst % NPC) // P           # 0..NB-1 within core
    group = block // GROUP_BLOCKS
    # cell id in stream order: (group, bank, block-within-group)
    cell = (group * BANKS + bank) * GROUP_BLOCKS + (block % GROUP_BLOCKS)
    ncells = NGROUPS * BANKS * GROUP_BLOCKS

    # sort by (core, cell, src): src-ascending within a cell gives the
    # random-row gather some HBM locality, and is required for nothing else
    order = np.lexsort((src, cell, core))
    core_s = core[order]
    cell_s = cell[order]
    src_s = src[order]
    dst_s = dst[order]

    counts = np.zeros((N_CORES, ncells), np.int64)
    flat = core * ncells + cell
    cnt = np.bincount(flat, minlength=N_CORES * ncells)
    counts = cnt.reshape(N_CORES, ncells)

    tiles = (counts + P - 1) // P
    tiles_u = tiles.max(axis=0)         # uniform per-cell tile count
    # guarantee every dst block has >=1 tile in its first present bank: give
    # bank 0's cell a minimum of one tile when the whole block is empty.
    tpc = np.zeros((BANKS, NB), np.int64)
    for c in range(ncells):
        g, b, dd = c // (BANKS * GROUP_BLOCKS), (c // GROUP_BLOCKS) % BANKS, c % GROUP_BLOCKS
        tpc[b][g * GROUP_BLOCKS + dd] = tiles_u[c]
    for d in range(NB):
        if tpc[:, d].sum() == 0:
            tpc[0][d] = 1
            g, dd = d // GROUP_BLOCKS, d % GROUP_BLOCKS
            tiles_u[(g * BANKS + 0) * GROUP_BLOCKS + dd] = 1

    slots = tiles_u * P
    cell_base = np.zeros(ncells + 1, np.int64)
    np.cumsum(slots, out=cell_base[1:])
    L = int(cell_base[-1])              # padded edge-slot count per core
    LT = L // P

    # position of each (sorted) edge inside its core's padded stream
    core_starts = np.searchsorted(core_s, np.arange(N_CORES + 1))
    # padding slots start at idx=-1; all but the call-trailing ones are
    # rewritten to 0 below (the gather ucode only trims TRAILING negatives;
    # a mid-stream negative would be fetched at base-256, out of bounds)
    idx_all = np.full((N_CORES, L), -1, np.int64)    # bank-local src index
    dv_all = np.full((N_CORES, L), -1.0, np.float32)  # block-local dst or -1
    for c in range(N_CORES):
        lo, hi = core_starts[c], core_starts[c + 1]
        cells_c = cell_s[lo:hi]
        # rank within cell
        starts = np.searchsorted(cells_c, np.arange(ncells))
        rank = np.arange(hi - lo) - starts[cells_c]
        pos = cell_base[cells_c] + rank
        idx_all[c][pos] = src_s[lo:hi] % BANK_ROWS
        dv_all[c][pos] = (dst_s[lo:hi] % NPC) % P

    # every gather call ends at a (group, bank) boundary, i.e. at the end of
    # the group's LAST block-cell; only those cells' pads are call-trailing
    # and may stay -1 (trimmed for free).  All other pads become idx 0.
    cell_of_slot = np.searchsorted(cell_base[1:], np.arange(L), side="right")
    nonfinal = (cell_of_slot % GROUP_BLOCKS) != GROUP_BLOCKS - 1
    for c in range(N_CORES):
        idx_all[c][(idx_all[c] < 0)] = 0  # BISECT: trim disabled

    f8np = mybir.dt.np(F8)
    per_core = []
    for c in range(N_CORES):
        idx_wrapped = np.tile(
            idx_all[c].astype(np.int16).reshape(L // 16, 16).T, (8, 1))
        dvw = dv_all[c].reshape(LT, P).T  # [P, LT]
        # one-hot scatter matrices in fp8 (1.0 = 0x38); pads (dv=-1) stay 0
        smat_u8 = np.zeros((P, LT, P), np.uint8)
        pp, tt = np.nonzero(dvw >= 0)
        smat_u8[pp, tt, dvw[pp, tt].astype(np.int64)] = 0x38
        per_core.append((idx_wrapped, smat_u8.view(f8np)))

    deg_in_sh = [deg_in[c * NPC:(c + 1) * NPC].reshape(NB, P).T.copy()
                 for c in range(N_CORES)]
    deg_out_sh = [deg_out[c * NPC:(c + 1) * NPC].reshape(NB, P).T.copy()
                  for c in range(N_CORES)]
    return tpc.tolist(), per_core, deg_in_sh, deg_out_sh


# --------------------------------------------------------------------------
# entry point
# --------------------------------------------------------------------------

def kernel(feat, W1, b1, W2, b2, src, dst):
    global LAST_EXEC_NS
    LAST_EXEC_NS = []
    _install_ntff_shim()

    feat = np.asarray(feat, np.float32)
    W1 = np.asarray(W1, np.float32)
    W2 = np.asarray(W2, np.float32)
    b1 = np.asarray(b1, np.float32)
    b2 = np.asarray(b2, np.float32)
    src = np.asarray(src)
    dst = np.asarray(dst)

    t0 = time.time()
    tpc, per_core, deg_in_sh, deg_out_sh = _preprocess(src, dst)
    featp = np.zeros((N, D), np.float32)
    featp[:N_RAW] = feat
    ident = np.eye(P, dtype=np.float32)
    b1r = np.tile(b1, (P, 1))
    b2r = np.tile(b2, (P, 1))
    if TRACE:
        print(f"[kernel] preprocess {time.time()-t0:.1f}s", flush=True)

    key = tuple(tuple(r) for r in tpc)
    if "dense" not in _cache:
        t = time.time()
        _cache["dense"] = _build_dense()
        if TRACE:
            print(f"[kernel] dense build+compile {time.time()-t:.1f}s", flush=True)
    if ("layer", key) not in _cache:
        t = time.time()
        _cache[("layer", key)] = _build_layer(tpc)
        if TRACE:
            print(f"[kernel] layer build+compile {time.time()-t:.1f}s", flush=True)
    nc_dense = _cache["dense"]
    nc_layer = _cache[("layer", key)]

    run_kw = dict(core_ids=list(range(N_CORES)), trace=TRACE)

    # ---- stage A: t1 = (feat * norm_s) @ W1 ----
    in_maps = [{
        "feat": featp[c * NPC:(c + 1) * NPC],
        "deg_out": deg_out_sh[c],
        "w1": W1, "ident": ident,
    } for c in range(N_CORES)]
    res = bass_utils.run_bass_kernel_spmd(nc_dense, in_maps, **run_kw)
    LAST_EXEC_NS.append(("dense_t1", res.exec_time_ns))
    t1_full = np.concatenate(
        [res.results[c]["t1t"].T for c in range(N_CORES)], axis=0)

    # ---- stage B/C: the two GCN layers ----
    table = np.ascontiguousarray(t1_full)
    out_shards = None
    for li, (bias_r, wnext) in enumerate([(b1r, W2), (b2r, W2)]):
        in_maps = [{
            "table": table,
            "idx16": per_core[c][0], "smat": per_core[c][1],
            "deg_in": deg_in_sh[c], "deg_out": deg_out_sh[c],
            "wn": wnext, "bias": bias_r, "ident": ident,
        } for c in range(N_CORES)]
        res = bass_utils.run_bass_kernel_spmd(nc_layer, in_maps, **run_kw)
        LAST_EXEC_NS.append((f"layer{li + 1}", res.exec_time_ns))
        if li == 0:
            table = np.ascontiguousarray(np.concatenate(
                [res.results[c]["tnt"].T for c in range(N_CORES)], axis=0))
        else:
            out_shards = [res.results[c]["out1"] for c in range(N_CORES)]

    out = np.concatenate(out_shards, axis=0)[:N_RAW]
    if TRACE:
        print(f"[kernel] total wall {time.time()-t0:.1f}s  exec: {LAST_EXEC_NS}",
              flush=True)
    return out.astype(np.float32)

